# revision 48
# baseline (speedup 1.0000x reference)
"""Trainium2 Bass kernel for a transformer MiniBlock (B=4, T=2048, C=1024, 16 heads,
causal attention, 4x FFN), sharded over 8 NeuronCores.

Sharding: core = (batch b=core//2, role r=core%2). Each core runs the full block for
1024 of its batch's 2048 tokens (four 256-token chunks, balanced for causal work),
computing k/v over the full sequence (no cross-core communication).

v3 (pipeline rewrite):
 - Phase 1 slice-pipelined: per 512-token slice, LN1 stats/outs overlap the
   previous slice's q/k/v projection matmuls; all projection weights preloaded.
 - Attention: the two parity score psums are merged into one [128,1024] 2-bank
   psum per k-tile-pair, so ONE Exp covers all four quarters (halves ACT op
   count).  AV matmuls for pair p are issued interleaved with the score
   matmuls of pair p+1 (software pipelining), so the tensor queue never
   head-blocks on the score->exp->AV dependency chain.
 - Softmax normalization fused: an = av_psum * bc_psum directly on DVE (no
   intermediate copies), masks applied as two [128,512] DVE ops per group.
 - FFN unchanged from v2 (it already ran at peak PE rate), LN2 ahead of it.

The program is SPMD-uniform: the q-column offsets use the role-0 chunk set; for
role-1 cores the host swaps adjacent 256-token chunks of xT (0<->1, 2<->3, ...)
so the program's query columns hold the role-1 chunks, and ships causal masks
built in that permuted k-coordinate system.
"""
import sys

sys.path.insert(0, "/opt/trn_rl_repo")

import numpy as np
from contextlib import ExitStack

import concourse.bacc as bacc
import concourse.mybir as mybir
import concourse.tile as tile

F32 = mybir.dt.float32
BF16 = mybir.dt.bfloat16
AF = mybir.ActivationFunctionType
ALU = mybir.AluOpType

P = 128
T = 2048          # full sequence
C = 1024          # embedding
NQ = 1024         # query tokens per core
H4 = 4096         # ffn hidden
NPAIR = 8         # head pairs
KC = C // P       # 8 channel tiles
NSLOT = 4         # 256-token query chunks per core
CW = 256          # chunk width
NKT_PROG = [4, 16, 8, 12]          # k-tiles per slot (program constant, max over roles)
GSETS = [[0, 7, 2, 5], [1, 6, 3, 4]]  # global 256-chunk index per slot, per role
LN_EPS = 1e-5
SCALE = 64.0 ** -0.5  # 0.125 (folded into Wq on host)


def _build():
    nc = bacc.Bacc(None, target_bir_lowering=False, debug=False)
    names = {}
    with tile.TileContext(nc) as tc, ExitStack() as top:
        dram = top.enter_context(tc.tile_pool(name="io", bufs=1, space="DRAM"))

        def din(name, shape, dt=BF16):
            t = dram.tile(shape, dt, kind="ExternalInput", name=f"i_{name}")
            names[name] = t.name
            return t

        xT_d = din("xT", [C, T])                  # raw x, channel-major
        Wq_d = din("Wq", [C, C])                  # scale*g1 folded
        Wk_d = din("Wk", [C, C])                  # g1 folded
        Wv_d = din("Wv", [C, C])                  # g1 folded
        Wo_d = din("Wo", [C, C])
        W1_d = din("W1", [C, H4])                 # g2 folded
        W2_d = din("W2", [H4, C])
        bqt_d = din("bqt", [P, NPAIR], F32)       # q bias cols (scale*ln1_b@Wq)
        bot_d = din("bot", [P, KC], F32)          # bo + (ln1_b@Wv)@Wo
        b1t_d = din("b1t", [P, 32], F32)          # b1 + ln2_b@W1
        b2t_d = din("b2t", [P, KC], F32)
        onesC_d = din("onesC", [P, 1])            # bf16 ones column
        onesB_d = din("onesB", [1, P])            # bf16 ones row
        masks_d = din("masks", [P, NSLOT, 2, 512])  # causal masks per k-tile-pair

        out_d = dram.tile([C, NQ], F32, kind="ExternalOutput", name="o_out")
        names["out"] = out_d.name

        # ---- persistent small sbuf ----
        pers = top.enter_context(tc.tile_pool(name="pers", bufs=1))
        onesC = pers.tile([P, 1], BF16, tag="onesC")
        nc.sync.dma_start(out=onesC[:], in_=onesC_d[:])
        onesB = pers.tile([1, P], BF16, tag="onesB")
        nc.sync.dma_start(out=onesB[:], in_=onesB_d[:])
        bqt = pers.tile([P, NPAIR], F32, tag="bqt")
        nc.sync.dma_start(out=bqt[:], in_=bqt_d[:])
        bot = pers.tile([P, KC], F32, tag="bot")
        nc.sync.dma_start(out=bot[:], in_=bot_d[:])
        b1t = pers.tile([P, 32], F32, tag="b1t")
        nc.sync.dma_start(out=b1t[:], in_=b1t_d[:])
        b2t = pers.tile([P, KC], F32, tag="b2t")
        nc.sync.dma_start(out=b2t[:], in_=b2t_d[:])
        eps_col = pers.tile([P, 1], F32, tag="eps_col")
        nc.vector.memset(eps_col[:], LN_EPS)

        def wstream(pool, wd, m, tag):
            """[P, KC, P] stationary tile: all contraction k-tiles of W[:, m*P:(m+1)*P]."""
            w = pool.tile([P, KC, P], BF16, tag=tag, name=tag)
            nc.sync.dma_start(
                out=w[:], in_=wd[:, m * P:(m + 1) * P]
                .rearrange("(kc p) m -> p kc m", p=P))
            return w

        # x2T survives into the FFN phase; everything else in phases 1+2 is
        # scoped inside ph12 so FFN gets the SBUF back.
        p_x2t = top.enter_context(tc.tile_pool(name="p_x2t", bufs=1))
        ph12 = top.enter_context(ExitStack())
        p_kT = ph12.enter_context(tc.tile_pool(name="p_kT", bufs=1))
        p_qT = ph12.enter_context(tc.tile_pool(name="p_qT", bufs=1))
        p_v = ph12.enter_context(tc.tile_pool(name="p_v", bufs=1))
        p_xq = ph12.enter_context(tc.tile_pool(name="p_xq", bufs=1))

        kT = [p_kT.tile([P, T], BF16, tag=f"kT{m}", name=f"kT{m}")
              for m in range(NPAIR)]
        qT = {}
        xq = [p_xq.tile([P, NQ], BF16, tag=f"xq{kc}", name=f"xq{kc}")
              for kc in range(KC)]
        v65 = []
        for tt in range(T // P):
            vt = p_v.tile([P, NPAIR, 2, 65], BF16, tag=f"v65_{tt}",
                          name=f"v65_{tt}")
            nc.vector.memset(vt[:, :, :, 64:65], 1.0)
            v65.append(vt)

        masks = []  # filled during phase 1 (DMA queued once xT/Wk are in flight)

        # slot index of the chunk living in t-slice i (role-0 program coords)
        slot_of_slice = {}
        for s, g in enumerate(GSETS[0]):
            slot_of_slice[g // 2] = (s, g)

        # =====================================================================
        # Phase 1: LN1 + q/k/v projections, slice-pipelined (4 x 512 cols)
        # =====================================================================
        with ExitStack() as ph1:
            wps = ph1.enter_context(tc.tile_pool(name="wps", bufs=1, side="right"))
            xt_pool = ph1.enter_context(tc.tile_pool(name="xt", bufs=2, side="right"))
            ln_pool = ph1.enter_context(tc.tile_pool(name="ln", bufs=2, side="right"))
            lnw = ph1.enter_context(tc.tile_pool(name="lnw", bufs=2, side="right"))
            pstat = ph1.enter_context(tc.tile_pool(name="pstat", bufs=1, space="PSUM"))
            pproj = ph1.enter_context(tc.tile_pool(name="pproj", bufs=4, space="PSUM"))

            def dma_x(i):
                xt = []
                for kc in range(KC):
                    t = xt_pool.tile([P, 512], BF16, tag=f"xT{kc}", name=f"xT{kc}")
                    nc.sync.dma_start(
                        out=t[:], in_=xT_d[kc * P:(kc + 1) * P,
                                           i * 512:(i + 1) * 512])
                    xt.append(t)
                return xt

            def ln_stats(xt):
                ps_sum = pstat.tile([1, 512], F32, tag="lns", name="ps_sum")
                ps_sq = pstat.tile([1, 512], F32, tag="lnq", name="ps_sq")
                for kc in range(KC):
                    sq = lnw.tile([P, 512], BF16, tag="sq", name="sq")
                    nc.vector.tensor_tensor(sq[:], xt[kc][:], xt[kc][:], ALU.mult)
                    nc.tensor.matmul(ps_sum[:], onesC[:], xt[kc][:],
                                     start=(kc == 0), stop=(kc == KC - 1),
                                     skip_group_check=True)
                    nc.tensor.matmul(ps_sq[:], onesC[:], sq[:],
                                     start=(kc == 0), stop=(kc == KC - 1),
                                     skip_group_check=True)
                return ps_sum, ps_sq

            def ln_rows(stats):
                """Stat psums -> mean / mean-square bf16 rows (ACT)."""
                ps_sum, ps_sq = stats
                mu_row = lnw.tile([1, 512], BF16, tag="mu_row", name="mu_row")
                msq_row = lnw.tile([1, 512], BF16, tag="msq_row", name="msq_row")
                nc.scalar.activation(mu_row[:], ps_sum[:], AF.Copy, scale=1.0 / C)
                nc.scalar.activation(msq_row[:], ps_sq[:], AF.Copy, scale=1.0 / C)
                return mu_row, msq_row

            def ln_rest(rows, xt):
                """Broadcast rows -> var/rstd -> normalized bf16 tiles."""
                mu_row, msq_row = rows
                mu_b = lnw.tile([P, 512], BF16, tag="mu_b", name="mu_b")
                nc.gpsimd.partition_broadcast(mu_b[:], mu_row[:])
                msq_b = lnw.tile([P, 512], BF16, tag="msq_b", name="msq_b")
                nc.gpsimd.partition_broadcast(msq_b[:], msq_row[:])
                var = lnw.tile([P, 512], F32, tag="var", name="var")
                nc.vector.tensor_tensor(var[:], mu_b[:], mu_b[:], ALU.mult)
                nc.vector.tensor_tensor(var[:], msq_b[:], var[:], ALU.subtract)
                nc.scalar.activation(var[:], var[:], AF.Sqrt, bias=eps_col[:])
                rstd_f = lnw.tile([P, 512], F32, tag="rstd_f", name="rstd_f")
                nc.vector.reciprocal_approx_fast(rstd_f[:], var[:])
                rstd_b = lnw.tile([P, 512], BF16, tag="rstd_b", name="rstd_b")
                nc.vector.tensor_copy(rstd_b[:], rstd_f[:])
                ln1 = []
                for kc in range(KC):
                    o = ln_pool.tile([P, 512], BF16, tag=f"ln{kc}", name=f"ln{kc}")
                    nc.vector.tensor_tensor(o[:], xt[kc][:], mu_b[:],
                                            ALU.subtract)
                    nc.vector.tensor_tensor(o[:], o[:], rstd_b[:], ALU.mult)
                    ln1.append(o)
                return ln1

            # prologue: first slice fully through LN (head of the pipeline)
            xt_cur = dma_x(0)
            stats0 = ln_stats(xt_cur)

            # Wk/Wq stationaries are streamed per slice (bufs=4 rotation);
            # only Wv stays resident (its DMA is issued inside iteration 0 so
            # it doesn't delay the first wkm stream in the DMA queue).
            wvn = []
            wkq_pool = ph1.enter_context(
                tc.tile_pool(name="wkq", bufs=4, side="right"))

            ln_cur = ln_rest(ln_rows(stats0), xt_cur)
            for i in range(4):
                ln1, xt_i = ln_cur, xt_cur
                sl = slice(i * 512, (i + 1) * 512)
                s_i, g_i = slot_of_slice[i]
                lo = g_i * CW - i * 512

                def kproj(ms):
                    for m in ms:
                        wkm = wstream(wkq_pool, Wk_d, m, "wkm")
                        ps = pproj.tile([P, 512], F32, tag="proj", name="kps")
                        for kc in range(KC):
                            nc.tensor.matmul(ps[:], wkm[:, kc, :], ln1[kc][:],
                                             start=(kc == 0), stop=(kc == KC - 1))
                        nc.scalar.activation(kT[m][:, sl], ps[:], AF.Copy)

                # interleave slice-(i+1) LN prefetch into the middle of the
                # projection stream: stats matmuls land after kproj(0..3) so
                # the tensor engine never waits on the sq DVE ops, and the
                # ln_rest broadcasts land before q/v so ln(i+1) is ready in
                # time for iteration i+1.
                kproj(range(4))
                if i == 1:
                    masks_t = pers.tile([P, NSLOT, 2, 512], BF16, tag="masks")
                    nc.sync.dma_start(out=masks_t[:], in_=masks_d[:])
                    masks.append(masks_t)
                if i == 0:
                    for n in range(2):
                        w = wps.tile([P, KC, 512], BF16, tag=f"wvn{n}",
                                     name=f"wvn{n}")
                        nc.sync.dma_start(
                            out=w[:], in_=Wv_d[:, n * 512:(n + 1) * 512]
                            .rearrange("(kc p) m -> p kc m", p=P))
                        wvn.append(w)
                rows_next = None
                if i < 3:
                    xt_cur = dma_x(i + 1)
                    rows_next = ln_rows(ln_stats(xt_cur))
                kproj(range(4, NPAIR))
                if i < 3:
                    ln_cur = ln_rest(rows_next, xt_cur)

                # residual query columns for the chunk living in this slice
                for kc in range(KC):
                    nc.vector.tensor_scalar(
                        xq[kc][:, s_i * CW:(s_i + 1) * CW],
                        xt_i[kc][:, lo:lo + CW],
                        bot[:, kc:kc + 1], None, ALU.add)

                # q projection for this slice's chunk (bias applied on ACT)
                for m in range(NPAIR):
                    wqm = wstream(wkq_pool, Wq_d, m, "wqm")
                    ps = pproj.tile([P, 512], F32, tag="proj", name="qps")
                    for kc in range(KC):
                        nc.tensor.matmul(ps[:, 0:CW], wqm[:, kc, :],
                                         ln1[kc][:, lo:lo + CW],
                                         start=(kc == 0), stop=(kc == KC - 1))
                    qt = p_qT.tile([P, CW], BF16, tag=f"qT{m}_{s_i}",
                                   name=f"qT{m}_{s_i}")
                    nc.scalar.activation(qt[:], ps[:, 0:CW], AF.Identity,
                                         bias=bqt[:, m:m + 1])
                    qT[(m, s_i)] = qt

                # v projection, token-major, for this slice's 4 token tiles
                for tl in range(4):
                    tt = 4 * i + tl
                    for n in range(2):
                        ps = pproj.tile([P, 512], F32, tag="proj", name="vps")
                        for kc in range(KC):
                            nc.tensor.matmul(
                                ps[:], ln1[kc][:, tl * P:(tl + 1) * P],
                                wvn[n][:, kc, :],
                                start=(kc == 0), stop=(kc == KC - 1))
                        nc.scalar.activation(
                            v65[tt][:, 4 * n:4 * (n + 1), :, 0:64],
                            ps[:].rearrange("p (pr par d) -> p pr par d",
                                            pr=4, par=2), AF.Copy)

        # =====================================================================
        # Phase 2: attention, pair-level software pipeline
        # =====================================================================
        with ExitStack() as ph2:
            x2T = [p_x2t.tile([P, NQ], BF16, tag=f"x2t{kc}", name=f"x2t{kc}")
                   for kc in range(KC)]
            wop = ph2.enter_context(tc.tile_pool(name="wop", bufs=1, side="right"))
            wo = [wstream(wop, Wo_d, mc, f"wom{mc}") for mc in range(KC)]
            pt_pool = ph2.enter_context(tc.tile_pool(name="pt", bufs=3))
            an_pool = ph2.enter_context(tc.tile_pool(name="an", bufs=2))
            sm_pool = ph2.enter_context(tc.tile_pool(name="sm", bufs=2))
            pSC = ph2.enter_context(tc.tile_pool(name="pSC", bufs=2, space="PSUM"))
            pAV2 = ph2.enter_context(tc.tile_pool(name="pAV2", bufs=3, space="PSUM"))
            pWO = ph2.enter_context(tc.tile_pool(name="pWO", bufs=1, space="PSUM"))

            # interleave a mask-heavy slot (0 or 2: half/all of their k-tile
            # groups carry masks -> DVE-heavy rounds) with a mask-light one
            # (1 or 3) so the per-round DVE load stays under the exp pace.
            tasks = []
            for hi, lo in ((1, 0), (3, 2)):
                for m in range(NPAIR):
                    tasks += [(hi, m), (lo, m)]
            task_of = {sm: t for t, sm in enumerate(tasks)}
            state = {}   # task idx -> dict(pt=[...], av=tile, s=, m=)

            def issue_scores(t, kt2):
                s, m = tasks[t]
                psc = pSC.tile([P, 1024], F32, tag="sc", name="psc")
                qe = qT[(m, s)]
                for j in range(2):
                    kws = slice((2 * kt2 + j) * P, (2 * kt2 + j + 1) * P)
                    nc.tensor.matmul(psc[:, j * CW:(j + 1) * CW],
                                     kT[m][0:64, kws], qe[0:64, :],
                                     start=True, stop=True,
                                     skip_group_check=True)
                    nc.tensor.matmul(psc[:, 512 + j * CW:512 + (j + 1) * CW],
                                     kT[m][64:128, kws], qe[64:128, :],
                                     start=True, stop=True,
                                     skip_group_check=True)
                pt = pt_pool.tile([P, 1024], BF16, tag=f"pt{kt2}",
                                  name=f"pt{kt2}")
                nc.scalar.activation(pt[:], psc[:], AF.Exp)
                nkt2 = NKT_PROG[s] // 2
                if kt2 >= nkt2 - 2:
                    g = kt2 - (nkt2 - 2)
                    mt = masks[0][:, s, g, :]
                    nc.vector.tensor_tensor(pt[:, 0:512], pt[:, 0:512], mt,
                                            ALU.mult)
                    nc.vector.tensor_tensor(pt[:, 512:1024], pt[:, 512:1024],
                                            mt, ALU.mult)
                state[t]["pt"].append(pt)

            def issue_av(t, par, kt2):
                # av_e (par=0) and av_o (par=1) share one psum bank, split by
                # column.  A start=True matmul marks its whole 2KB zero-region
                # pending, so the two accumulation groups must NOT interleave:
                # par=0 runs to completion before par=1's start (verified on
                # hw: reads of pending-but-unwritten bytes see old data, but
                # accumulates onto them lose the prior value).
                s, m = tasks[t]
                st = state[t]
                nkt2 = NKT_PROG[s] // 2
                av = st["av"]
                pt = st["pt"][kt2]
                for j in range(2):
                    kt = 2 * kt2 + j
                    b = (kt2 == 0 and j == 0)
                    e = (kt2 == nkt2 - 1 and j == 1)
                    nc.tensor.matmul(av[:, par * CW:(par + 1) * CW],
                                     v65[kt][:, m, par, :],
                                     pt[:, par * 512 + j * CW:
                                        par * 512 + (j + 1) * CW],
                                     start=b, stop=e, skip_group_check=True)

            def issue_normalize(t):
                s, m = tasks[t]
                av = state[t]["av"]
                an = an_pool.tile([P, CW], BF16, tag=f"an{m}", name=f"an{m}")
                state[t]["an"] = an
                den = sm_pool.tile([1, 512], F32, tag="den", name="den")
                nc.vector.tensor_copy(den[:], av[64:65, :])
                rec = sm_pool.tile([1, 512], F32, tag="rec", name="rec")
                nc.vector.reciprocal_approx_fast(rec[:], den[:])
                bcb = sm_pool.tile([64, 512], F32, tag="bcb", name="bcb")
                for par in range(2):
                    cs = slice(par * CW, (par + 1) * CW)
                    nc.gpsimd.partition_broadcast(bcb[:, cs], rec[:, cs])
                nc.vector.tensor_tensor(an[0:64, :], av[0:64, 0:CW],
                                        bcb[:, 0:CW], ALU.mult)
                tmo = sm_pool.tile([64, CW], BF16, tag="tmo", name="tmo")
                nc.vector.tensor_tensor(tmo[:], av[0:64, CW:2 * CW],
                                        bcb[:, CW:2 * CW], ALU.mult)
                nc.sync.dma_start(out=an[64:128, :], in_=tmo[:])

            def issue_wo_chain(mc, s):
                ps = pWO.tile([P, CW], F32, tag="wo", name="wops")
                for k in range(NPAIR):
                    nc.tensor.matmul(ps[:], wo[mc][:, k, :],
                                     state[task_of[(s, k)]]["an"][:],
                                     start=(k == 0), stop=(k == NPAIR - 1))
                nc.vector.tensor_tensor(x2T[mc][:, s * CW:(s + 1) * CW],
                                        ps[:], xq[mc][:, s * CW:(s + 1) * CW],
                                        ALU.add)

            def finish_task(t):
                """AV (odd parity) + normalize for task t; returns Wo work
                if t closed a slot."""
                pn2 = NKT_PROG[tasks[t][0]] // 2
                for kt2 in range(pn2):
                    issue_av(t, 1, kt2)
                issue_normalize(t)
                if tasks[t][1] == NPAIR - 1:
                    return [(mc, tasks[t][0]) for mc in range(KC)]
                return []

            # AV lags scores by TWO tasks so the exp pipeline stays full even
            # on short (nkt=4) slots; Wo chains are spread one-per-round so
            # slot boundaries don't drain the exp stream.
            NT = len(tasks)
            pending_wo = []
            for t in range(NT):
                s, m = tasks[t]
                nkt2 = NKT_PROG[s] // 2
                state[t] = {"pt": [], "av": pAV2.tile([65, 512], F32, tag="av",
                                                      name="av")}
                pn = NKT_PROG[tasks[t - 2][0]] // 2 if t >= 2 else 0
                for kt2 in range(max(nkt2, pn)):
                    if kt2 < nkt2:
                        issue_scores(t, kt2)
                    if t >= 2 and kt2 < pn:
                        issue_av(t - 2, 0, kt2)
                    if pending_wo and kt2 % 2 == 1:
                        issue_wo_chain(*pending_wo.pop(0))
                if t >= 2:
                    pending_wo += finish_task(t - 2)
            for t in (NT - 2, NT - 1):
                for kt2 in range(NKT_PROG[tasks[t][0]] // 2):
                    issue_av(t, 0, kt2)
                pending_wo += finish_task(t)
                while pending_wo:
                    issue_wo_chain(*pending_wo.pop(0))

        # =====================================================================
        # Phase 3: LN2 + FFN (gamma2 folded into W1, ln2_b into b1)
        # =====================================================================
        with ExitStack() as ph5:
            p_ln2T = ph5.enter_context(tc.tile_pool(name="p_ln2T", bufs=1))
            ln2stack = ExitStack()
            lnw2 = ln2stack.enter_context(tc.tile_pool(name="lnw2", bufs=2))
            pstat2 = ln2stack.enter_context(tc.tile_pool(name="pstat2", bufs=2,
                                                         space="PSUM"))
            ln2T = [p_ln2T.tile([P, NQ], BF16, tag=f"ln2T{kc}", name=f"ln2T{kc}")
                    for kc in range(KC)]
            for i in range(NQ // 512):
                sl = slice(i * 512, (i + 1) * 512)
                ps_sum = pstat2.tile([1, 512], F32, tag="lns", name="ps_sum")
                ps_sq = pstat2.tile([1, 512], F32, tag="lnq", name="ps_sq")
                for kc in range(KC):
                    sq = lnw2.tile([P, 512], BF16, tag="sq", name="sq")
                    nc.vector.tensor_tensor(sq[:], x2T[kc][:, sl], x2T[kc][:, sl],
                                            ALU.mult)
                    nc.tensor.matmul(ps_sum[:], onesC[:], x2T[kc][:, sl],
                                     start=(kc == 0), stop=(kc == KC - 1),
                                     skip_group_check=True)
                    nc.tensor.matmul(ps_sq[:], onesC[:], sq[:],
                                     start=(kc == 0), stop=(kc == KC - 1),
                                     skip_group_check=True)
                mu_row = lnw2.tile([1, 512], BF16, tag="mu_row", name="mu_row")
                msq_row = lnw2.tile([1, 512], BF16, tag="msq_row", name="msq_row")
                nc.scalar.activation(mu_row[:], ps_sum[:], AF.Copy, scale=1.0 / C)
                nc.scalar.activation(msq_row[:], ps_sq[:], AF.Copy, scale=1.0 / C)
                mu_b = lnw2.tile([P, 512], BF16, tag="mu_b", name="mu_b")
                nc.gpsimd.partition_broadcast(mu_b[:], mu_row[:])
                msq_b = lnw2.tile([P, 512], BF16, tag="msq_b", name="msq_b")
                nc.gpsimd.partition_broadcast(msq_b[:], msq_row[:])
                var = lnw2.tile([P, 512], F32, tag="var", name="var")
                nc.vector.tensor_tensor(var[:], mu_b[:], mu_b[:], ALU.mult)
                nc.vector.tensor_tensor(var[:], msq_b[:], var[:], ALU.subtract)
                nc.scalar.activation(var[:], var[:], AF.Sqrt, bias=eps_col[:])
                rstd_f = lnw2.tile([P, 512], F32, tag="rstd_f", name="rstd_f")
                nc.vector.reciprocal_approx_fast(rstd_f[:], var[:])
                rstd_b = lnw2.tile([P, 512], BF16, tag="rstd_b", name="rstd_b")
                nc.vector.tensor_copy(rstd_b[:], rstd_f[:])
                for kc in range(KC):
                    nc.vector.tensor_tensor(ln2T[kc][:, sl], x2T[kc][:, sl],
                                            mu_b[:], ALU.subtract)
                    nc.vector.tensor_tensor(ln2T[kc][:, sl], ln2T[kc][:, sl],
                                            rstd_b[:], ALU.mult)
            ln2stack.close()

            ff1_pool = ph5.enter_context(tc.tile_pool(name="ff1", bufs=1))
            facc_pool = ph5.enter_context(tc.tile_pool(name="facc", bufs=1))
            w1_pool = ph5.enter_context(tc.tile_pool(name="w1s", bufs=3))
            w2_pool = ph5.enter_context(tc.tile_pool(name="w2s", bufs=2))
            fst_pool = ph5.enter_context(tc.tile_pool(name="fst", bufs=3))
            pF = ph5.enter_context(tc.tile_pool(name="pF", bufs=4, space="PSUM"))
            ffacc = [facc_pool.tile([P, NQ], BF16, tag=f"facc{m}", name=f"ffacc{m}")
                     for m in range(KC)]
            for half in range(2):
                hoff = half * 2048
                ff1 = []
                for m in range(16):
                    mm = half * 16 + m
                    w1m = w1_pool.tile([P, KC, P], BF16, tag="w1m", name="w1m")
                    nc.sync.dma_start(
                        out=w1m[:],
                        in_=W1_d[:, hoff + m * P: hoff + (m + 1) * P]
                        .rearrange("(kc p) m -> p kc m", p=P))
                    f = ff1_pool.tile([P, NQ], BF16, tag=f"f{m}", name=f"f{m}")
                    for tch in range(2):
                        sl = slice(tch * 512, (tch + 1) * 512)
                        psf = pF.tile([P, 512], F32, tag="proj", name="f1ps")
                        for kc in range(KC):
                            nc.tensor.matmul(psf[:], w1m[:, kc, :], ln2T[kc][:, sl],
                                             start=(kc == 0), stop=(kc == KC - 1))
                        nc.scalar.activation(f[:, sl], psf[:], AF.Relu,
                                             bias=b1t[:, mm:mm + 1])
                    ff1.append(f)
                for mc in range(KC):
                    w2m = w2_pool.tile([P, 16, P], BF16, tag="w2m", name="w2m")
                    nc.sync.dma_start(
                        out=w2m[:],
                        in_=W2_d[hoff:hoff + 2048, mc * P:(mc + 1) * P]
                        .rearrange("(kt p) m -> p kt m", p=P))
                    for tch in range(2):
                        sl = slice(tch * 512, (tch + 1) * 512)
                        psf = pF.tile([P, 512], F32, tag="proj", name="f2ps")
                        for kt in range(16):
                            nc.tensor.matmul(psf[:], w2m[:, kt, :], ff1[kt][:, sl],
                                             start=(kt == 0), stop=(kt == 15))
                        if half == 0:
                            nc.vector.tensor_scalar(ffacc[mc][:, sl], psf[:],
                                                    b2t[:, mc:mc + 1], None,
                                                    ALU.add)
                        else:
                            o = fst_pool.tile([P, 512], F32, tag="fo", name="fo")
                            nc.vector.tensor_tensor(o[:], psf[:], ffacc[mc][:, sl],
                                                    ALU.add)
                            nc.vector.tensor_tensor(o[:], o[:], x2T[mc][:, sl],
                                                    ALU.add)
                            nc.sync.dma_start(out=out_d[mc * P:(mc + 1) * P, sl],
                                              in_=o[:])

    nc.compile()
    return nc, names


_CACHE = {}


def _get_built():
    if "nc" not in _CACHE:
        _CACHE["nc"], _CACHE["names"] = _build()
    return _CACHE["nc"], _CACHE["names"]


def _bf16(a):
    import ml_dtypes
    return np.ascontiguousarray(np.asarray(a).astype(ml_dtypes.bfloat16))


# role-1 cores get xT with adjacent 256-token chunks swapped, so that the
# program's role-0 query columns hold the role-1 chunks.  chunk_at[p] = global
# chunk stored at program chunk position p.
_CHUNK_AT = {0: list(range(8)), 1: [1, 0, 3, 2, 5, 4, 7, 6]}


def _host_inputs(x, Wq, Wk, Wv, Wo, bo, ln1_g, ln1_b, ln2_g, ln2_b, W1, b1, W2, b2):
    """Build the 8 per-core input maps (host work = sharding/layout + affine
    weight folding)."""
    f = np.float32
    g1 = ln1_g.astype(f)[:, None]
    Wq_f = SCALE * g1 * Wq.astype(f)
    Wk_f = g1 * Wk.astype(f)
    Wv_f = g1 * Wv.astype(f)
    bq = SCALE * (ln1_b.astype(f) @ Wq.astype(f))          # q bias (applied)
    bv = ln1_b.astype(f) @ Wv.astype(f)                    # v bias -> folds into bo
    bo_f = bo.astype(f) + bv @ Wo.astype(f)
    g2 = ln2_g.astype(f)[:, None]
    W1_f = g2 * W1.astype(f)
    b1_f = b1.astype(f) + ln2_b.astype(f) @ W1.astype(f)

    shared = {
        "Wq": _bf16(Wq_f), "Wk": _bf16(Wk_f), "Wv": _bf16(Wv_f),
        "Wo": _bf16(Wo.astype(f)), "W1": _bf16(W1_f), "W2": _bf16(W2.astype(f)),
        "bqt": np.ascontiguousarray(bq.reshape(NPAIR, P).T, f),
        "bot": np.ascontiguousarray(bo_f.reshape(KC, P).T, f),
        "b1t": np.ascontiguousarray(b1_f.reshape(32, P).T, f),
        "b2t": np.ascontiguousarray(b2.astype(f).reshape(KC, P).T, f),
        "onesC": _bf16(np.ones((P, 1), f)),
        "onesB": _bf16(np.ones((1, P), f)),
    }
    kl = np.arange(P)[:, None]
    ql = np.arange(CW)[None, :]
    in_maps = []
    for c in range(8):
        b, r = c // 2, c % 2
        chunk_at = _CHUNK_AT[r]
        xTb = np.ascontiguousarray(x[b].T.astype(f))  # (C, T)
        if r == 1:
            cols = np.concatenate([np.arange(CW * pc, CW * (pc + 1))
                                   for pc in chunk_at])
            xTb = np.ascontiguousarray(xTb[:, cols])
        # causal masks in program k-coordinates: program k position j lives in
        # program chunk j//CW, which holds global chunk chunk_at[j//CW]; its
        # global index is chunk_at[j//CW]*CW + j%CW.  Query slot s holds global
        # chunk GSETS[r][s].  Layout: [s, g, 0:256]=tile nkt-4+2g,
        # [s, g, 256:512]=tile nkt-4+2g+1.
        m = np.empty((NSLOT, 2, P, 512), f)
        for s in range(NSLOT):
            q_glob0 = CW * GSETS[r][s]
            for g in range(2):
                for j in range(2):
                    kt = NKT_PROG[s] - 4 + 2 * g + j
                    kpos = P * kt + kl                      # [P,1] program index
                    kglob = (np.asarray(chunk_at)[kpos // CW] * CW) + (kpos % CW)
                    m[s, g, :, j * CW:(j + 1) * CW] = \
                        (kglob <= (q_glob0 + ql)).astype(f)
        im = dict(shared)
        im["xT"] = _bf16(xTb)
        im["masks"] = _bf16(np.ascontiguousarray(m.transpose(2, 0, 1, 3)))
        in_maps.append(im)
    return in_maps


def _unshard(outs):
    out = np.empty((4, T, C), np.float32)
    for c in range(8):
        b, r = c // 2, c % 2
        oT = outs[c]  # (C, NQ), program slot order
        for s in range(NSLOT):
            g = GSETS[r][s]
            out[b, CW * g:CW * (g + 1), :] = oT[:, CW * s:CW * (s + 1)].T
    return out


def kernel(**inputs):
    from concourse.bass_utils import run_bass_kernel_spmd
    from concourse.bass_interp import get_hw_module

    args = {k: np.asarray(v, np.float32) for k, v in inputs.items()}
    in_maps_named = _host_inputs(**args)

    nc, names = _get_built()
    in_maps = [{names[k]: v for k, v in im.items()} for im in in_maps_named]

    hw = get_hw_module(nc.m)
    old = nc.m
    nc.m = hw
    try:
        res = run_bass_kernel_spmd(nc, in_maps, core_ids=list(range(8)))
    finally:
        nc.m = old
    outs = [r[names["out"]] for r in res.results]
    return _unshard(outs)


if __name__ == "__main__":
    import reference
    inp = {k: np.asarray(v) for k, v in reference.setup_inputs().items()}
    got = kernel(**inp)
    exp = np.asarray(reference.reference(**inp))
    err = np.abs(got - exp).max() / np.abs(exp).max()
    print("Relative error:", err)


# revision 49
# speedup vs baseline: 1.0162x; 1.0162x over previous
"""Trainium2 Bass kernel for a transformer MiniBlock (B=4, T=2048, C=1024, 16 heads,
causal attention, 4x FFN), sharded over 8 NeuronCores.

Sharding: core = (batch b=core//2, role r=core%2). Each core runs the full block for
1024 of its batch's 2048 tokens (four 256-token chunks, balanced for causal work),
computing k/v over the full sequence (no cross-core communication).

v3 (pipeline rewrite):
 - Phase 1 slice-pipelined: per 512-token slice, LN1 stats/outs overlap the
   previous slice's q/k/v projection matmuls; all projection weights preloaded.
 - Attention: the two parity score psums are merged into one [128,1024] 2-bank
   psum per k-tile-pair, so ONE Exp covers all four quarters (halves ACT op
   count).  AV matmuls for pair p are issued interleaved with the score
   matmuls of pair p+1 (software pipelining), so the tensor queue never
   head-blocks on the score->exp->AV dependency chain.
 - Softmax normalization fused: an = av_psum * bc_psum directly on DVE (no
   intermediate copies), masks applied as two [128,512] DVE ops per group.
 - FFN unchanged from v2 (it already ran at peak PE rate), LN2 ahead of it.

The program is SPMD-uniform: the q-column offsets use the role-0 chunk set; for
role-1 cores the host swaps adjacent 256-token chunks of xT (0<->1, 2<->3, ...)
so the program's query columns hold the role-1 chunks, and ships causal masks
built in that permuted k-coordinate system.
"""
import sys

sys.path.insert(0, "/opt/trn_rl_repo")

import numpy as np
from contextlib import ExitStack

import concourse.bacc as bacc
import concourse.mybir as mybir
import concourse.tile as tile

F32 = mybir.dt.float32
BF16 = mybir.dt.bfloat16
AF = mybir.ActivationFunctionType
ALU = mybir.AluOpType

P = 128
T = 2048          # full sequence
C = 1024          # embedding
NQ = 1024         # query tokens per core
H4 = 4096         # ffn hidden
NPAIR = 8         # head pairs
KC = C // P       # 8 channel tiles
NSLOT = 4         # 256-token query chunks per core
CW = 256          # chunk width
NKT_PROG = [4, 16, 8, 12]          # k-tiles per slot (program constant, max over roles)
GSETS = [[0, 7, 2, 5], [1, 6, 3, 4]]  # global 256-chunk index per slot, per role
LN_EPS = 1e-5
SCALE = 64.0 ** -0.5  # 0.125 (folded into Wq on host)


def _build():
    nc = bacc.Bacc(None, target_bir_lowering=False, debug=False)
    names = {}
    with tile.TileContext(nc) as tc, ExitStack() as top:
        dram = top.enter_context(tc.tile_pool(name="io", bufs=1, space="DRAM"))

        def din(name, shape, dt=BF16):
            t = dram.tile(shape, dt, kind="ExternalInput", name=f"i_{name}")
            names[name] = t.name
            return t

        xT_d = din("xT", [C, T])                  # raw x, channel-major
        Wq_d = din("Wq", [C, C])                  # scale*g1 folded
        Wk_d = din("Wk", [C, C])                  # g1 folded
        Wv_d = din("Wv", [C, C])                  # g1 folded
        Wo_d = din("Wo", [C, C])
        W1_d = din("W1", [C, H4])                 # g2 folded
        W2_d = din("W2", [H4, C])
        bqt_d = din("bqt", [P, NPAIR], F32)       # q bias cols (scale*ln1_b@Wq)
        bot_d = din("bot", [P, KC], F32)          # bo + (ln1_b@Wv)@Wo
        b1t_d = din("b1t", [P, 32], F32)          # b1 + ln2_b@W1
        b2t_d = din("b2t", [P, KC], F32)
        onesC_d = din("onesC", [P, 1])            # bf16 ones column
        onesB_d = din("onesB", [1, P])            # bf16 ones row
        masks_d = din("masks", [P, NSLOT, 2, 512])  # causal masks per k-tile-pair

        out_d = dram.tile([C, NQ], F32, kind="ExternalOutput", name="o_out")
        names["out"] = out_d.name

        # ---- persistent small sbuf ----
        pers = top.enter_context(tc.tile_pool(name="pers", bufs=1))
        onesC = pers.tile([P, 1], BF16, tag="onesC")
        nc.sync.dma_start(out=onesC[:], in_=onesC_d[:])
        onesB = pers.tile([1, P], BF16, tag="onesB")
        nc.sync.dma_start(out=onesB[:], in_=onesB_d[:])
        bqt = pers.tile([P, NPAIR], F32, tag="bqt")
        nc.sync.dma_start(out=bqt[:], in_=bqt_d[:])
        bot = pers.tile([P, KC], F32, tag="bot")
        nc.sync.dma_start(out=bot[:], in_=bot_d[:])
        b1t = pers.tile([P, 32], F32, tag="b1t")
        nc.sync.dma_start(out=b1t[:], in_=b1t_d[:])
        b2t = pers.tile([P, KC], F32, tag="b2t")
        nc.sync.dma_start(out=b2t[:], in_=b2t_d[:])
        eps_col = pers.tile([P, 1], F32, tag="eps_col")
        nc.vector.memset(eps_col[:], LN_EPS)

        def wstream(pool, wd, m, tag):
            """[P, KC, P] stationary tile: all contraction k-tiles of W[:, m*P:(m+1)*P]."""
            w = pool.tile([P, KC, P], BF16, tag=tag, name=tag)
            nc.sync.dma_start(
                out=w[:], in_=wd[:, m * P:(m + 1) * P]
                .rearrange("(kc p) m -> p kc m", p=P))
            return w

        # x2T survives into the FFN phase; everything else in phases 1+2 is
        # scoped inside ph12 so FFN gets the SBUF back.
        p_x2t = top.enter_context(tc.tile_pool(name="p_x2t", bufs=1))
        ph12 = top.enter_context(ExitStack())
        p_kT = ph12.enter_context(tc.tile_pool(name="p_kT", bufs=1))
        p_qT = ph12.enter_context(tc.tile_pool(name="p_qT", bufs=1))
        p_v = ph12.enter_context(tc.tile_pool(name="p_v", bufs=1))
        p_xq = ph12.enter_context(tc.tile_pool(name="p_xq", bufs=1))

        kT = [p_kT.tile([P, T], BF16, tag=f"kT{m}", name=f"kT{m}")
              for m in range(NPAIR)]
        qT = {}
        xq = [p_xq.tile([P, NQ], BF16, tag=f"xq{kc}", name=f"xq{kc}")
              for kc in range(KC)]
        v65 = []
        for tt in range(T // P):
            vt = p_v.tile([P, NPAIR, 2, 65], BF16, tag=f"v65_{tt}",
                          name=f"v65_{tt}")
            nc.vector.memset(vt[:, :, :, 64:65], 1.0)
            v65.append(vt)

        masks = []  # filled during phase 1 (DMA queued once xT/Wk are in flight)

        # slot index of the chunk living in t-slice i (role-0 program coords)
        slot_of_slice = {}
        for s, g in enumerate(GSETS[0]):
            slot_of_slice[g // 2] = (s, g)

        # =====================================================================
        # Phase 1: LN1 + q/k/v projections, slice-pipelined (4 x 512 cols)
        # =====================================================================
        with ExitStack() as ph1:
            wps = ph1.enter_context(tc.tile_pool(name="wps", bufs=1, side="right"))
            xt_pool = ph1.enter_context(tc.tile_pool(name="xt", bufs=2, side="right"))
            ln_pool = ph1.enter_context(tc.tile_pool(name="ln", bufs=2, side="right"))
            lnw = ph1.enter_context(tc.tile_pool(name="lnw", bufs=2, side="right"))
            pstat = ph1.enter_context(tc.tile_pool(name="pstat", bufs=1, space="PSUM"))
            pproj = ph1.enter_context(tc.tile_pool(name="pproj", bufs=4, space="PSUM"))

            def dma_x(i):
                xt = []
                for kc in range(KC):
                    t = xt_pool.tile([P, 512], BF16, tag=f"xT{kc}", name=f"xT{kc}")
                    nc.sync.dma_start(
                        out=t[:], in_=xT_d[kc * P:(kc + 1) * P,
                                           i * 512:(i + 1) * 512])
                    xt.append(t)
                return xt

            def ln_stats(xt):
                ps_sum = pstat.tile([1, 512], F32, tag="lns", name="ps_sum")
                ps_sq = pstat.tile([1, 512], F32, tag="lnq", name="ps_sq")
                for kc in range(KC):
                    sq = lnw.tile([P, 512], BF16, tag="sq", name="sq")
                    nc.vector.tensor_tensor(sq[:], xt[kc][:], xt[kc][:], ALU.mult)
                    nc.tensor.matmul(ps_sum[:], onesC[:], xt[kc][:],
                                     start=(kc == 0), stop=(kc == KC - 1),
                                     skip_group_check=True)
                    nc.tensor.matmul(ps_sq[:], onesC[:], sq[:],
                                     start=(kc == 0), stop=(kc == KC - 1),
                                     skip_group_check=True)
                return ps_sum, ps_sq

            def ln_rows(stats):
                """Stat psums -> mean / mean-square bf16 rows (ACT)."""
                ps_sum, ps_sq = stats
                mu_row = lnw.tile([1, 512], BF16, tag="mu_row", name="mu_row")
                msq_row = lnw.tile([1, 512], BF16, tag="msq_row", name="msq_row")
                nc.scalar.activation(mu_row[:], ps_sum[:], AF.Copy, scale=1.0 / C)
                nc.scalar.activation(msq_row[:], ps_sq[:], AF.Copy, scale=1.0 / C)
                return mu_row, msq_row

            def ln_rest(rows, xt):
                """Broadcast rows -> var/rstd -> normalized bf16 tiles."""
                mu_row, msq_row = rows
                mu_b = lnw.tile([P, 512], BF16, tag="mu_b", name="mu_b")
                nc.gpsimd.partition_broadcast(mu_b[:], mu_row[:])
                msq_b = lnw.tile([P, 512], BF16, tag="msq_b", name="msq_b")
                nc.gpsimd.partition_broadcast(msq_b[:], msq_row[:])
                var = lnw.tile([P, 512], F32, tag="var", name="var")
                nc.vector.tensor_tensor(var[:], mu_b[:], mu_b[:], ALU.mult)
                nc.vector.tensor_tensor(var[:], msq_b[:], var[:], ALU.subtract)
                nc.scalar.activation(var[:], var[:], AF.Sqrt, bias=eps_col[:])
                rstd_f = lnw.tile([P, 512], F32, tag="rstd_f", name="rstd_f")
                nc.vector.reciprocal_approx_fast(rstd_f[:], var[:])
                rstd_b = lnw.tile([P, 512], BF16, tag="rstd_b", name="rstd_b")
                nc.vector.tensor_copy(rstd_b[:], rstd_f[:])
                ln1 = []
                for kc in range(KC):
                    o = ln_pool.tile([P, 512], BF16, tag=f"ln{kc}", name=f"ln{kc}")
                    nc.vector.tensor_tensor(o[:], xt[kc][:], mu_b[:],
                                            ALU.subtract)
                    nc.vector.tensor_tensor(o[:], o[:], rstd_b[:], ALU.mult)
                    ln1.append(o)
                return ln1

            # prologue: first slice fully through LN (head of the pipeline)
            xt_cur = dma_x(0)
            stats0 = ln_stats(xt_cur)

            # Wk/Wq stationaries are streamed per slice (bufs=4 rotation);
            # only Wv stays resident (its DMA is issued inside iteration 0 so
            # it doesn't delay the first wkm stream in the DMA queue).
            wvn = []
            wkq_pool = ph1.enter_context(
                tc.tile_pool(name="wkq", bufs=4, side="right"))

            ln_cur = ln_rest(ln_rows(stats0), xt_cur)
            for i in range(4):
                ln1, xt_i = ln_cur, xt_cur
                sl = slice(i * 512, (i + 1) * 512)
                s_i, g_i = slot_of_slice[i]
                lo = g_i * CW - i * 512

                def kproj(ms):
                    for m in ms:
                        wkm = wstream(wkq_pool, Wk_d, m, "wkm")
                        ps = pproj.tile([P, 512], F32, tag="proj", name="kps")
                        for kc in range(KC):
                            nc.tensor.matmul(ps[:], wkm[:, kc, :], ln1[kc][:],
                                             start=(kc == 0), stop=(kc == KC - 1))
                        nc.scalar.activation(kT[m][:, sl], ps[:], AF.Copy)

                # interleave slice-(i+1) LN prefetch into the middle of the
                # projection stream: stats matmuls land after kproj(0..3) so
                # the tensor engine never waits on the sq DVE ops, and the
                # ln_rest broadcasts land before q/v so ln(i+1) is ready in
                # time for iteration i+1.
                kproj(range(4))
                if i == 1:
                    masks_t = pers.tile([P, NSLOT, 2, 512], BF16, tag="masks")
                    nc.sync.dma_start(out=masks_t[:], in_=masks_d[:])
                    masks.append(masks_t)
                if i == 0:
                    for n in range(2):
                        w = wps.tile([P, KC, 512], BF16, tag=f"wvn{n}",
                                     name=f"wvn{n}")
                        nc.sync.dma_start(
                            out=w[:], in_=Wv_d[:, n * 512:(n + 1) * 512]
                            .rearrange("(kc p) m -> p kc m", p=P))
                        wvn.append(w)
                rows_next = None
                if i < 3:
                    xt_cur = dma_x(i + 1)
                    rows_next = ln_rows(ln_stats(xt_cur))
                kproj(range(4, NPAIR))
                if i < 3:
                    ln_cur = ln_rest(rows_next, xt_cur)

                # residual query columns for the chunk living in this slice
                for kc in range(KC):
                    nc.vector.tensor_scalar(
                        xq[kc][:, s_i * CW:(s_i + 1) * CW],
                        xt_i[kc][:, lo:lo + CW],
                        bot[:, kc:kc + 1], None, ALU.add)

                # q projection for this slice's chunk (bias applied on ACT)
                for m in range(NPAIR):
                    wqm = wstream(wkq_pool, Wq_d, m, "wqm")
                    ps = pproj.tile([P, 512], F32, tag="proj", name="qps")
                    for kc in range(KC):
                        nc.tensor.matmul(ps[:, 0:CW], wqm[:, kc, :],
                                         ln1[kc][:, lo:lo + CW],
                                         start=(kc == 0), stop=(kc == KC - 1))
                    qt = p_qT.tile([P, CW], BF16, tag=f"qT{m}_{s_i}",
                                   name=f"qT{m}_{s_i}")
                    nc.scalar.activation(qt[:], ps[:, 0:CW], AF.Identity,
                                         bias=bqt[:, m:m + 1])
                    qT[(m, s_i)] = qt

                # v projection, token-major, for this slice's 4 token tiles
                for tl in range(4):
                    tt = 4 * i + tl
                    for n in range(2):
                        ps = pproj.tile([P, 512], F32, tag="proj", name="vps")
                        for kc in range(KC):
                            nc.tensor.matmul(
                                ps[:], ln1[kc][:, tl * P:(tl + 1) * P],
                                wvn[n][:, kc, :],
                                start=(kc == 0), stop=(kc == KC - 1))
                        nc.scalar.activation(
                            v65[tt][:, 4 * n:4 * (n + 1), :, 0:64],
                            ps[:].rearrange("p (pr par d) -> p pr par d",
                                            pr=4, par=2), AF.Copy)

        # =====================================================================
        # Phase 2: attention, pair-level software pipeline
        # =====================================================================
        with ExitStack() as ph2:
            x2T = [p_x2t.tile([P, NQ], BF16, tag=f"x2t{kc}", name=f"x2t{kc}")
                   for kc in range(KC)]
            wop = ph2.enter_context(tc.tile_pool(name="wop", bufs=1, side="right"))
            wo = [wstream(wop, Wo_d, mc, f"wom{mc}") for mc in range(KC)]
            pt_pool = ph2.enter_context(tc.tile_pool(name="pt", bufs=3))
            an_pool = ph2.enter_context(tc.tile_pool(name="an", bufs=2))
            sm_pool = ph2.enter_context(tc.tile_pool(name="sm", bufs=2))
            pSC = ph2.enter_context(tc.tile_pool(name="pSC", bufs=2, space="PSUM"))
            pAV2 = ph2.enter_context(tc.tile_pool(name="pAV2", bufs=3, space="PSUM"))
            pWO = ph2.enter_context(tc.tile_pool(name="pWO", bufs=1, space="PSUM"))

            # interleave a mask-heavy slot (0 or 2: half/all of their k-tile
            # groups carry masks -> DVE-heavy rounds) with a mask-light one
            # (1 or 3) so the per-round DVE load stays under the exp pace.
            tasks = []
            for hi, lo in ((1, 0), (3, 2)):
                for m in range(NPAIR):
                    tasks += [(hi, m), (lo, m)]
            task_of = {sm: t for t, sm in enumerate(tasks)}
            state = {}   # task idx -> dict(pt=[...], av=tile, s=, m=)

            def issue_scores(t, kt2):
                s, m = tasks[t]
                psc = pSC.tile([P, 1024], F32, tag="sc", name="psc")
                qe = qT[(m, s)]
                for j in range(2):
                    kws = slice((2 * kt2 + j) * P, (2 * kt2 + j + 1) * P)
                    nc.tensor.matmul(psc[:, j * CW:(j + 1) * CW],
                                     kT[m][0:64, kws], qe[0:64, :],
                                     start=True, stop=True,
                                     skip_group_check=True)
                    nc.tensor.matmul(psc[:, 512 + j * CW:512 + (j + 1) * CW],
                                     kT[m][64:128, kws], qe[64:128, :],
                                     start=True, stop=True,
                                     skip_group_check=True)
                pt = pt_pool.tile([P, 1024], BF16, tag=f"pt{kt2}",
                                  name=f"pt{kt2}")
                nc.scalar.activation(pt[:], psc[:], AF.Exp)
                nkt2 = NKT_PROG[s] // 2
                if kt2 >= nkt2 - 2:
                    g = kt2 - (nkt2 - 2)
                    mt = masks[0][:, s, g, :]
                    nc.vector.tensor_tensor(pt[:, 0:512], pt[:, 0:512], mt,
                                            ALU.mult)
                    nc.vector.tensor_tensor(pt[:, 512:1024], pt[:, 512:1024],
                                            mt, ALU.mult)
                state[t]["pt"].append(pt)

            def issue_av(t, par, kt2):
                # av_e (par=0) and av_o (par=1) share one psum bank, split by
                # column.  A start=True matmul marks its whole 2KB zero-region
                # pending, so the two accumulation groups must NOT interleave:
                # par=0 runs to completion before par=1's start (verified on
                # hw: reads of pending-but-unwritten bytes see old data, but
                # accumulates onto them lose the prior value).
                s, m = tasks[t]
                st = state[t]
                nkt2 = NKT_PROG[s] // 2
                av = st["av"]
                pt = st["pt"][kt2]
                for j in range(2):
                    kt = 2 * kt2 + j
                    b = (kt2 == 0 and j == 0)
                    e = (kt2 == nkt2 - 1 and j == 1)
                    nc.tensor.matmul(av[:, par * CW:(par + 1) * CW],
                                     v65[kt][:, m, par, :],
                                     pt[:, par * 512 + j * CW:
                                        par * 512 + (j + 1) * CW],
                                     start=b, stop=e, skip_group_check=True)

            def issue_normalize(t):
                s, m = tasks[t]
                av = state[t]["av"]
                an = an_pool.tile([P, CW], BF16, tag=f"an{m}", name=f"an{m}")
                state[t]["an"] = an
                den = sm_pool.tile([1, 512], F32, tag="den", name="den")
                nc.vector.tensor_copy(den[:], av[64:65, :])
                rec = sm_pool.tile([1, 512], F32, tag="rec", name="rec")
                nc.vector.reciprocal_approx_fast(rec[:], den[:])
                recb = sm_pool.tile([1, 512], BF16, tag="recb", name="recb")
                nc.vector.tensor_copy(recb[:], rec[:])
                bcb = sm_pool.tile([64, 512], BF16, tag="bcb", name="bcb")
                for par in range(2):
                    cs = slice(par * CW, (par + 1) * CW)
                    nc.gpsimd.partition_broadcast(bcb[:, cs], recb[:, cs])
                nc.vector.tensor_tensor(an[0:64, :], av[0:64, 0:CW],
                                        bcb[:, 0:CW], ALU.mult)
                tmo = sm_pool.tile([64, CW], BF16, tag="tmo", name="tmo")
                nc.vector.tensor_tensor(tmo[:], av[0:64, CW:2 * CW],
                                        bcb[:, CW:2 * CW], ALU.mult)
                nc.sync.dma_start(out=an[64:128, :], in_=tmo[:])

            def issue_wo_chain(mc, s):
                ps = pWO.tile([P, CW], F32, tag="wo", name="wops")
                for k in range(NPAIR):
                    nc.tensor.matmul(ps[:], wo[mc][:, k, :],
                                     state[task_of[(s, k)]]["an"][:],
                                     start=(k == 0), stop=(k == NPAIR - 1))
                nc.vector.tensor_tensor(x2T[mc][:, s * CW:(s + 1) * CW],
                                        ps[:], xq[mc][:, s * CW:(s + 1) * CW],
                                        ALU.add)

            def finish_task(t):
                """AV (odd parity) + normalize for task t; returns Wo work
                if t closed a slot."""
                pn2 = NKT_PROG[tasks[t][0]] // 2
                for kt2 in range(pn2):
                    issue_av(t, 1, kt2)
                issue_normalize(t)
                if tasks[t][1] == NPAIR - 1:
                    return [(mc, tasks[t][0]) for mc in range(KC)]
                return []

            # AV lags scores by TWO tasks so the exp pipeline stays full even
            # on short (nkt=4) slots; Wo chains are spread one-per-round so
            # slot boundaries don't drain the exp stream.
            NT = len(tasks)
            pending_wo = []
            for t in range(NT):
                s, m = tasks[t]
                nkt2 = NKT_PROG[s] // 2
                state[t] = {"pt": [], "av": pAV2.tile([65, 512], F32, tag="av",
                                                      name="av")}
                pn = NKT_PROG[tasks[t - 2][0]] // 2 if t >= 2 else 0
                for kt2 in range(max(nkt2, pn)):
                    if kt2 < nkt2:
                        issue_scores(t, kt2)
                    if t >= 2 and kt2 < pn:
                        issue_av(t - 2, 0, kt2)
                    if pending_wo and kt2 % 2 == 1:
                        issue_wo_chain(*pending_wo.pop(0))
                if t >= 2:
                    pending_wo += finish_task(t - 2)
            for t in (NT - 2, NT - 1):
                for kt2 in range(NKT_PROG[tasks[t][0]] // 2):
                    issue_av(t, 0, kt2)
                pending_wo += finish_task(t)
                while pending_wo:
                    issue_wo_chain(*pending_wo.pop(0))

        # =====================================================================
        # Phase 3: LN2 + FFN (gamma2 folded into W1, ln2_b into b1)
        # =====================================================================
        with ExitStack() as ph5:
            p_ln2T = ph5.enter_context(tc.tile_pool(name="p_ln2T", bufs=1))
            ln2stack = ExitStack()
            lnw2 = ln2stack.enter_context(tc.tile_pool(name="lnw2", bufs=2))
            pstat2 = ln2stack.enter_context(tc.tile_pool(name="pstat2", bufs=2,
                                                         space="PSUM"))
            ln2T = [p_ln2T.tile([P, NQ], BF16, tag=f"ln2T{kc}", name=f"ln2T{kc}")
                    for kc in range(KC)]
            for i in range(NQ // 512):
                sl = slice(i * 512, (i + 1) * 512)
                ps_sum = pstat2.tile([1, 512], F32, tag="lns", name="ps_sum")
                ps_sq = pstat2.tile([1, 512], F32, tag="lnq", name="ps_sq")
                for kc in range(KC):
                    sq = lnw2.tile([P, 512], BF16, tag="sq", name="sq")
                    nc.vector.tensor_tensor(sq[:], x2T[kc][:, sl], x2T[kc][:, sl],
                                            ALU.mult)
                    nc.tensor.matmul(ps_sum[:], onesC[:], x2T[kc][:, sl],
                                     start=(kc == 0), stop=(kc == KC - 1),
                                     skip_group_check=True)
                    nc.tensor.matmul(ps_sq[:], onesC[:], sq[:],
                                     start=(kc == 0), stop=(kc == KC - 1),
                                     skip_group_check=True)
                mu_row = lnw2.tile([1, 512], BF16, tag="mu_row", name="mu_row")
                msq_row = lnw2.tile([1, 512], BF16, tag="msq_row", name="msq_row")
                nc.scalar.activation(mu_row[:], ps_sum[:], AF.Copy, scale=1.0 / C)
                nc.scalar.activation(msq_row[:], ps_sq[:], AF.Copy, scale=1.0 / C)
                mu_b = lnw2.tile([P, 512], BF16, tag="mu_b", name="mu_b")
                nc.gpsimd.partition_broadcast(mu_b[:], mu_row[:])
                msq_b = lnw2.tile([P, 512], BF16, tag="msq_b", name="msq_b")
                nc.gpsimd.partition_broadcast(msq_b[:], msq_row[:])
                var = lnw2.tile([P, 512], F32, tag="var", name="var")
                nc.vector.tensor_tensor(var[:], mu_b[:], mu_b[:], ALU.mult)
                nc.vector.tensor_tensor(var[:], msq_b[:], var[:], ALU.subtract)
                nc.scalar.activation(var[:], var[:], AF.Sqrt, bias=eps_col[:])
                rstd_f = lnw2.tile([P, 512], F32, tag="rstd_f", name="rstd_f")
                nc.vector.reciprocal_approx_fast(rstd_f[:], var[:])
                rstd_b = lnw2.tile([P, 512], BF16, tag="rstd_b", name="rstd_b")
                nc.vector.tensor_copy(rstd_b[:], rstd_f[:])
                for kc in range(KC):
                    nc.vector.tensor_tensor(ln2T[kc][:, sl], x2T[kc][:, sl],
                                            mu_b[:], ALU.subtract)
                    nc.vector.tensor_tensor(ln2T[kc][:, sl], ln2T[kc][:, sl],
                                            rstd_b[:], ALU.mult)
            ln2stack.close()

            ff1_pool = ph5.enter_context(tc.tile_pool(name="ff1", bufs=1))
            facc_pool = ph5.enter_context(tc.tile_pool(name="facc", bufs=1))
            w1_pool = ph5.enter_context(tc.tile_pool(name="w1s", bufs=3))
            w2_pool = ph5.enter_context(tc.tile_pool(name="w2s", bufs=2))
            fst_pool = ph5.enter_context(tc.tile_pool(name="fst", bufs=3))
            pF = ph5.enter_context(tc.tile_pool(name="pF", bufs=4, space="PSUM"))
            ffacc = [facc_pool.tile([P, NQ], BF16, tag=f"facc{m}", name=f"ffacc{m}")
                     for m in range(KC)]
            for half in range(2):
                hoff = half * 2048
                ff1 = []
                for m in range(16):
                    mm = half * 16 + m
                    w1m = w1_pool.tile([P, KC, P], BF16, tag="w1m", name="w1m")
                    nc.sync.dma_start(
                        out=w1m[:],
                        in_=W1_d[:, hoff + m * P: hoff + (m + 1) * P]
                        .rearrange("(kc p) m -> p kc m", p=P))
                    f = ff1_pool.tile([P, NQ], BF16, tag=f"f{m}", name=f"f{m}")
                    for tch in range(2):
                        sl = slice(tch * 512, (tch + 1) * 512)
                        psf = pF.tile([P, 512], F32, tag="proj", name="f1ps")
                        for kc in range(KC):
                            nc.tensor.matmul(psf[:], w1m[:, kc, :], ln2T[kc][:, sl],
                                             start=(kc == 0), stop=(kc == KC - 1))
                        nc.scalar.activation(f[:, sl], psf[:], AF.Relu,
                                             bias=b1t[:, mm:mm + 1])
                    ff1.append(f)
                for mc in range(KC):
                    w2m = w2_pool.tile([P, 16, P], BF16, tag="w2m", name="w2m")
                    nc.sync.dma_start(
                        out=w2m[:],
                        in_=W2_d[hoff:hoff + 2048, mc * P:(mc + 1) * P]
                        .rearrange("(kt p) m -> p kt m", p=P))
                    for tch in range(2):
                        sl = slice(tch * 512, (tch + 1) * 512)
                        psf = pF.tile([P, 512], F32, tag="proj", name="f2ps")
                        for kt in range(16):
                            nc.tensor.matmul(psf[:], w2m[:, kt, :], ff1[kt][:, sl],
                                             start=(kt == 0), stop=(kt == 15))
                        if half == 0:
                            nc.vector.tensor_scalar(ffacc[mc][:, sl], psf[:],
                                                    b2t[:, mc:mc + 1], None,
                                                    ALU.add)
                        else:
                            o = fst_pool.tile([P, 512], F32, tag="fo", name="fo")
                            nc.vector.tensor_tensor(o[:], psf[:], ffacc[mc][:, sl],
                                                    ALU.add)
                            nc.vector.tensor_tensor(o[:], o[:], x2T[mc][:, sl],
                                                    ALU.add)
                            nc.sync.dma_start(out=out_d[mc * P:(mc + 1) * P, sl],
                                              in_=o[:])

    nc.compile()
    return nc, names


_CACHE = {}


def _get_built():
    if "nc" not in _CACHE:
        _CACHE["nc"], _CACHE["names"] = _build()
    return _CACHE["nc"], _CACHE["names"]


def _bf16(a):
    import ml_dtypes
    return np.ascontiguousarray(np.asarray(a).astype(ml_dtypes.bfloat16))


# role-1 cores get xT with adjacent 256-token chunks swapped, so that the
# program's role-0 query columns hold the role-1 chunks.  chunk_at[p] = global
# chunk stored at program chunk position p.
_CHUNK_AT = {0: list(range(8)), 1: [1, 0, 3, 2, 5, 4, 7, 6]}


def _host_inputs(x, Wq, Wk, Wv, Wo, bo, ln1_g, ln1_b, ln2_g, ln2_b, W1, b1, W2, b2):
    """Build the 8 per-core input maps (host work = sharding/layout + affine
    weight folding)."""
    f = np.float32
    g1 = ln1_g.astype(f)[:, None]
    Wq_f = SCALE * g1 * Wq.astype(f)
    Wk_f = g1 * Wk.astype(f)
    Wv_f = g1 * Wv.astype(f)
    bq = SCALE * (ln1_b.astype(f) @ Wq.astype(f))          # q bias (applied)
    bv = ln1_b.astype(f) @ Wv.astype(f)                    # v bias -> folds into bo
    bo_f = bo.astype(f) + bv @ Wo.astype(f)
    g2 = ln2_g.astype(f)[:, None]
    W1_f = g2 * W1.astype(f)
    b1_f = b1.astype(f) + ln2_b.astype(f) @ W1.astype(f)

    shared = {
        "Wq": _bf16(Wq_f), "Wk": _bf16(Wk_f), "Wv": _bf16(Wv_f),
        "Wo": _bf16(Wo.astype(f)), "W1": _bf16(W1_f), "W2": _bf16(W2.astype(f)),
        "bqt": np.ascontiguousarray(bq.reshape(NPAIR, P).T, f),
        "bot": np.ascontiguousarray(bo_f.reshape(KC, P).T, f),
        "b1t": np.ascontiguousarray(b1_f.reshape(32, P).T, f),
        "b2t": np.ascontiguousarray(b2.astype(f).reshape(KC, P).T, f),
        "onesC": _bf16(np.ones((P, 1), f)),
        "onesB": _bf16(np.ones((1, P), f)),
    }
    kl = np.arange(P)[:, None]
    ql = np.arange(CW)[None, :]
    in_maps = []
    for c in range(8):
        b, r = c // 2, c % 2
        chunk_at = _CHUNK_AT[r]
        xTb = np.ascontiguousarray(x[b].T.astype(f))  # (C, T)
        if r == 1:
            cols = np.concatenate([np.arange(CW * pc, CW * (pc + 1))
                                   for pc in chunk_at])
            xTb = np.ascontiguousarray(xTb[:, cols])
        # causal masks in program k-coordinates: program k position j lives in
        # program chunk j//CW, which holds global chunk chunk_at[j//CW]; its
        # global index is chunk_at[j//CW]*CW + j%CW.  Query slot s holds global
        # chunk GSETS[r][s].  Layout: [s, g, 0:256]=tile nkt-4+2g,
        # [s, g, 256:512]=tile nkt-4+2g+1.
        m = np.empty((NSLOT, 2, P, 512), f)
        for s in range(NSLOT):
            q_glob0 = CW * GSETS[r][s]
            for g in range(2):
                for j in range(2):
                    kt = NKT_PROG[s] - 4 + 2 * g + j
                    kpos = P * kt + kl                      # [P,1] program index
                    kglob = (np.asarray(chunk_at)[kpos // CW] * CW) + (kpos % CW)
                    m[s, g, :, j * CW:(j + 1) * CW] = \
                        (kglob <= (q_glob0 + ql)).astype(f)
        im = dict(shared)
        im["xT"] = _bf16(xTb)
        im["masks"] = _bf16(np.ascontiguousarray(m.transpose(2, 0, 1, 3)))
        in_maps.append(im)
    return in_maps


def _unshard(outs):
    out = np.empty((4, T, C), np.float32)
    for c in range(8):
        b, r = c // 2, c % 2
        oT = outs[c]  # (C, NQ), program slot order
        for s in range(NSLOT):
            g = GSETS[r][s]
            out[b, CW * g:CW * (g + 1), :] = oT[:, CW * s:CW * (s + 1)].T
    return out


def kernel(**inputs):
    from concourse.bass_utils import run_bass_kernel_spmd
    from concourse.bass_interp import get_hw_module

    args = {k: np.asarray(v, np.float32) for k, v in inputs.items()}
    in_maps_named = _host_inputs(**args)

    nc, names = _get_built()
    in_maps = [{names[k]: v for k, v in im.items()} for im in in_maps_named]

    hw = get_hw_module(nc.m)
    old = nc.m
    nc.m = hw
    try:
        res = run_bass_kernel_spmd(nc, in_maps, core_ids=list(range(8)))
    finally:
        nc.m = old
    outs = [r[names["out"]] for r in res.results]
    return _unshard(outs)


if __name__ == "__main__":
    import reference
    inp = {k: np.asarray(v) for k, v in reference.setup_inputs().items()}
    got = kernel(**inp)
    exp = np.asarray(reference.reference(**inp))
    err = np.abs(got - exp).max() / np.abs(exp).max()
    print("Relative error:", err)


# revision 51
# speedup vs baseline: 1.0219x; 1.0056x over previous
"""Trainium2 Bass kernel for a transformer MiniBlock (B=4, T=2048, C=1024, 16 heads,
causal attention, 4x FFN), sharded over 8 NeuronCores.

Sharding: core = (batch b=core//2, role r=core%2). Each core runs the full block for
1024 of its batch's 2048 tokens (four 256-token chunks, balanced for causal work),
computing k/v over the full sequence (no cross-core communication).

v3 (pipeline rewrite):
 - Phase 1 slice-pipelined: per 512-token slice, LN1 stats/outs overlap the
   previous slice's q/k/v projection matmuls; all projection weights preloaded.
 - Attention: the two parity score psums are merged into one [128,1024] 2-bank
   psum per k-tile-pair, so ONE Exp covers all four quarters (halves ACT op
   count).  AV matmuls for pair p are issued interleaved with the score
   matmuls of pair p+1 (software pipelining), so the tensor queue never
   head-blocks on the score->exp->AV dependency chain.
 - Softmax normalization fused: an = av_psum * bc_psum directly on DVE (no
   intermediate copies), masks applied as two [128,512] DVE ops per group.
 - FFN unchanged from v2 (it already ran at peak PE rate), LN2 ahead of it.

The program is SPMD-uniform: the q-column offsets use the role-0 chunk set; for
role-1 cores the host swaps adjacent 256-token chunks of xT (0<->1, 2<->3, ...)
so the program's query columns hold the role-1 chunks, and ships causal masks
built in that permuted k-coordinate system.
"""
import sys

sys.path.insert(0, "/opt/trn_rl_repo")

import numpy as np
from contextlib import ExitStack

import concourse.bacc as bacc
import concourse.mybir as mybir
import concourse.tile as tile

F32 = mybir.dt.float32
BF16 = mybir.dt.bfloat16
AF = mybir.ActivationFunctionType
ALU = mybir.AluOpType

P = 128
T = 2048          # full sequence
C = 1024          # embedding
NQ = 1024         # query tokens per core
H4 = 4096         # ffn hidden
NPAIR = 8         # head pairs
KC = C // P       # 8 channel tiles
NSLOT = 4         # 256-token query chunks per core
CW = 256          # chunk width
NKT_PROG = [4, 16, 8, 12]          # k-tiles per slot (program constant, max over roles)
GSETS = [[0, 7, 2, 5], [1, 6, 3, 4]]  # global 256-chunk index per slot, per role
LN_EPS = 1e-5
SCALE = 64.0 ** -0.5  # 0.125 (folded into Wq on host)


def _build():
    nc = bacc.Bacc(None, target_bir_lowering=False, debug=False)
    names = {}
    with tile.TileContext(nc) as tc, ExitStack() as top:
        dram = top.enter_context(tc.tile_pool(name="io", bufs=1, space="DRAM"))

        def din(name, shape, dt=BF16):
            t = dram.tile(shape, dt, kind="ExternalInput", name=f"i_{name}")
            names[name] = t.name
            return t

        xT_d = din("xT", [C, T])                  # raw x, channel-major
        Wq_d = din("Wq", [C, C])                  # scale*g1 folded
        Wk_d = din("Wk", [C, C])                  # g1 folded
        Wv_d = din("Wv", [C, C])                  # g1 folded
        Wo_d = din("Wo", [C, C])
        W1_d = din("W1", [C, H4])                 # g2 folded
        W2_d = din("W2", [H4, C])
        bqt_d = din("bqt", [P, NPAIR], F32)       # q bias cols (scale*ln1_b@Wq)
        bot_d = din("bot", [P, KC], F32)          # bo + (ln1_b@Wv)@Wo
        b1t_d = din("b1t", [P, 32], F32)          # b1 + ln2_b@W1
        b2t_d = din("b2t", [P, KC], F32)
        onesC_d = din("onesC", [P, 1])            # bf16 ones column
        onesB_d = din("onesB", [1, P])            # bf16 ones row
        masks_d = din("masks", [P, NSLOT, 2, 512])  # causal masks per k-tile-pair

        out_d = dram.tile([C, NQ], F32, kind="ExternalOutput", name="o_out")
        names["out"] = out_d.name

        # ---- persistent small sbuf ----
        pers = top.enter_context(tc.tile_pool(name="pers", bufs=1))
        onesC = pers.tile([P, 1], BF16, tag="onesC")
        nc.sync.dma_start(out=onesC[:], in_=onesC_d[:])
        onesB = pers.tile([1, P], BF16, tag="onesB")
        nc.sync.dma_start(out=onesB[:], in_=onesB_d[:])
        bqt = pers.tile([P, NPAIR], F32, tag="bqt")
        nc.sync.dma_start(out=bqt[:], in_=bqt_d[:])
        bot = pers.tile([P, KC], F32, tag="bot")
        nc.sync.dma_start(out=bot[:], in_=bot_d[:])
        b1t = pers.tile([P, 32], F32, tag="b1t")
        nc.sync.dma_start(out=b1t[:], in_=b1t_d[:])
        b2t = pers.tile([P, KC], F32, tag="b2t")
        nc.sync.dma_start(out=b2t[:], in_=b2t_d[:])
        eps_col = pers.tile([P, 1], F32, tag="eps_col")
        nc.vector.memset(eps_col[:], LN_EPS)

        def wstream(pool, wd, m, tag):
            """[P, KC, P] stationary tile: all contraction k-tiles of W[:, m*P:(m+1)*P]."""
            w = pool.tile([P, KC, P], BF16, tag=tag, name=tag)
            nc.sync.dma_start(
                out=w[:], in_=wd[:, m * P:(m + 1) * P]
                .rearrange("(kc p) m -> p kc m", p=P))
            return w

        # x2T survives into the FFN phase; everything else in phases 1+2 is
        # scoped inside ph12 so FFN gets the SBUF back.
        p_x2t = top.enter_context(tc.tile_pool(name="p_x2t", bufs=1))
        ph12 = top.enter_context(ExitStack())
        p_kT = ph12.enter_context(tc.tile_pool(name="p_kT", bufs=1))
        p_qT = ph12.enter_context(tc.tile_pool(name="p_qT", bufs=1))
        p_v = ph12.enter_context(tc.tile_pool(name="p_v", bufs=1))
        p_xq = ph12.enter_context(tc.tile_pool(name="p_xq", bufs=1))

        kT = [p_kT.tile([P, T], BF16, tag=f"kT{m}", name=f"kT{m}")
              for m in range(NPAIR)]
        qT = {}
        xq = [p_xq.tile([P, NQ], BF16, tag=f"xq{kc}", name=f"xq{kc}")
              for kc in range(KC)]
        v65 = []
        for tt in range(T // P):
            vt = p_v.tile([P, NPAIR, 2, 65], BF16, tag=f"v65_{tt}",
                          name=f"v65_{tt}")
            nc.vector.memset(vt[:, :, :, 64:65], 1.0)
            v65.append(vt)

        masks = []  # filled during phase 1 (DMA queued once xT/Wk are in flight)

        # slot index of the chunk living in t-slice i (role-0 program coords)
        slot_of_slice = {}
        for s, g in enumerate(GSETS[0]):
            slot_of_slice[g // 2] = (s, g)

        # =====================================================================
        # Phase 1: LN1 + q/k/v projections, slice-pipelined (4 x 512 cols)
        # =====================================================================
        with ExitStack() as ph1:
            wps = ph1.enter_context(tc.tile_pool(name="wps", bufs=1, side="right"))
            xt_pool = ph1.enter_context(tc.tile_pool(name="xt", bufs=2, side="right"))
            ln_pool = ph1.enter_context(tc.tile_pool(name="ln", bufs=2, side="right"))
            lnw = ph1.enter_context(tc.tile_pool(name="lnw", bufs=2, side="right"))
            pstat = ph1.enter_context(tc.tile_pool(name="pstat", bufs=1, space="PSUM"))
            pproj = ph1.enter_context(tc.tile_pool(name="pproj", bufs=4, space="PSUM"))

            def dma_x(i):
                xt = []
                for kc in range(KC):
                    t = xt_pool.tile([P, 512], BF16, tag=f"xT{kc}", name=f"xT{kc}")
                    nc.sync.dma_start(
                        out=t[:], in_=xT_d[kc * P:(kc + 1) * P,
                                           i * 512:(i + 1) * 512])
                    xt.append(t)
                return xt

            def ln_stats(xt):
                ps_sum = pstat.tile([1, 512], F32, tag="lns", name="ps_sum")
                ps_sq = pstat.tile([1, 512], F32, tag="lnq", name="ps_sq")
                for kc in range(KC):
                    sq = lnw.tile([P, 512], BF16, tag="sq", name="sq")
                    nc.vector.tensor_tensor(sq[:], xt[kc][:], xt[kc][:], ALU.mult)
                    nc.tensor.matmul(ps_sum[:], onesC[:], xt[kc][:],
                                     start=(kc == 0), stop=(kc == KC - 1),
                                     skip_group_check=True)
                    nc.tensor.matmul(ps_sq[:], onesC[:], sq[:],
                                     start=(kc == 0), stop=(kc == KC - 1),
                                     skip_group_check=True)
                return ps_sum, ps_sq

            def ln_rows(stats):
                """Stat psums -> mean / mean-square bf16 rows (ACT)."""
                ps_sum, ps_sq = stats
                mu_row = lnw.tile([1, 512], BF16, tag="mu_row", name="mu_row")
                msq_row = lnw.tile([1, 512], BF16, tag="msq_row", name="msq_row")
                nc.scalar.activation(mu_row[:], ps_sum[:], AF.Copy, scale=1.0 / C)
                nc.scalar.activation(msq_row[:], ps_sq[:], AF.Copy, scale=1.0 / C)
                return mu_row, msq_row

            def ln_rest(rows, xt):
                """Broadcast rows -> var/rstd -> normalized bf16 tiles."""
                mu_row, msq_row = rows
                mu_b = lnw.tile([P, 512], BF16, tag="mu_b", name="mu_b")
                nc.gpsimd.partition_broadcast(mu_b[:], mu_row[:])
                msq_b = lnw.tile([P, 512], BF16, tag="msq_b", name="msq_b")
                nc.gpsimd.partition_broadcast(msq_b[:], msq_row[:])
                var = lnw.tile([P, 512], F32, tag="var", name="var")
                nc.vector.tensor_tensor(var[:], mu_b[:], mu_b[:], ALU.mult)
                nc.vector.tensor_tensor(var[:], msq_b[:], var[:], ALU.subtract)
                nc.scalar.activation(var[:], var[:], AF.Sqrt, bias=eps_col[:])
                rstd_f = lnw.tile([P, 512], F32, tag="rstd_f", name="rstd_f")
                nc.vector.reciprocal_approx_fast(rstd_f[:], var[:])
                rstd_b = lnw.tile([P, 512], BF16, tag="rstd_b", name="rstd_b")
                nc.vector.tensor_copy(rstd_b[:], rstd_f[:])
                ln1 = []
                for kc in range(KC):
                    o = ln_pool.tile([P, 512], BF16, tag=f"ln{kc}", name=f"ln{kc}")
                    nc.vector.tensor_tensor(o[:], xt[kc][:], mu_b[:],
                                            ALU.subtract)
                    nc.vector.tensor_tensor(o[:], o[:], rstd_b[:], ALU.mult)
                    ln1.append(o)
                return ln1

            # prologue: first slice fully through LN (head of the pipeline)
            xt_cur = dma_x(0)
            stats0 = ln_stats(xt_cur)

            # Wk/Wq stationaries are streamed per slice (bufs=4 rotation);
            # only Wv stays resident (its DMA is issued inside iteration 0 so
            # it doesn't delay the first wkm stream in the DMA queue).
            wvn = []
            wkq_pool = ph1.enter_context(
                tc.tile_pool(name="wkq", bufs=4, side="right"))

            ln_cur = ln_rest(ln_rows(stats0), xt_cur)
            for i in range(4):
                ln1, xt_i = ln_cur, xt_cur
                sl = slice(i * 512, (i + 1) * 512)
                s_i, g_i = slot_of_slice[i]
                lo = g_i * CW - i * 512

                def kproj(ms):
                    for m in ms:
                        wkm = wstream(wkq_pool, Wk_d, m, "wkm")
                        ps = pproj.tile([P, 512], F32, tag="proj", name="kps")
                        for kc in range(KC):
                            nc.tensor.matmul(ps[:], wkm[:, kc, :], ln1[kc][:],
                                             start=(kc == 0), stop=(kc == KC - 1))
                        nc.scalar.activation(kT[m][:, sl], ps[:], AF.Copy)

                # interleave slice-(i+1) LN prefetch into the middle of the
                # projection stream: stats matmuls land after kproj(0..3) so
                # the tensor engine never waits on the sq DVE ops, and the
                # ln_rest broadcasts land before q/v so ln(i+1) is ready in
                # time for iteration i+1.
                kproj(range(4))
                if i == 1:
                    masks_t = pers.tile([P, NSLOT, 2, 512], BF16, tag="masks")
                    nc.sync.dma_start(out=masks_t[:], in_=masks_d[:])
                    masks.append(masks_t)
                if i == 0:
                    for n in range(2):
                        w = wps.tile([P, KC, 512], BF16, tag=f"wvn{n}",
                                     name=f"wvn{n}")
                        nc.sync.dma_start(
                            out=w[:], in_=Wv_d[:, n * 512:(n + 1) * 512]
                            .rearrange("(kc p) m -> p kc m", p=P))
                        wvn.append(w)
                rows_next = None
                if i < 3:
                    xt_cur = dma_x(i + 1)
                    rows_next = ln_rows(ln_stats(xt_cur))
                kproj(range(4, NPAIR))
                if i < 3:
                    ln_cur = ln_rest(rows_next, xt_cur)

                # residual query columns for the chunk living in this slice
                for kc in range(KC):
                    nc.vector.tensor_scalar(
                        xq[kc][:, s_i * CW:(s_i + 1) * CW],
                        xt_i[kc][:, lo:lo + CW],
                        bot[:, kc:kc + 1], None, ALU.add)

                # q projection for this slice's chunk (bias applied on ACT)
                for m in range(NPAIR):
                    wqm = wstream(wkq_pool, Wq_d, m, "wqm")
                    ps = pproj.tile([P, 512], F32, tag="proj", name="qps")
                    for kc in range(KC):
                        nc.tensor.matmul(ps[:, 0:CW], wqm[:, kc, :],
                                         ln1[kc][:, lo:lo + CW],
                                         start=(kc == 0), stop=(kc == KC - 1))
                    qt = p_qT.tile([P, CW], BF16, tag=f"qT{m}_{s_i}",
                                   name=f"qT{m}_{s_i}")
                    nc.scalar.activation(qt[:], ps[:, 0:CW], AF.Identity,
                                         bias=bqt[:, m:m + 1])
                    qT[(m, s_i)] = qt

                # v projection, token-major, for this slice's 4 token tiles
                for tl in range(4):
                    tt = 4 * i + tl
                    for n in range(2):
                        ps = pproj.tile([P, 512], F32, tag="proj", name="vps")
                        for kc in range(KC):
                            nc.tensor.matmul(
                                ps[:], ln1[kc][:, tl * P:(tl + 1) * P],
                                wvn[n][:, kc, :],
                                start=(kc == 0), stop=(kc == KC - 1))
                        nc.scalar.activation(
                            v65[tt][:, 4 * n:4 * (n + 1), :, 0:64],
                            ps[:].rearrange("p (pr par d) -> p pr par d",
                                            pr=4, par=2), AF.Copy)

        # =====================================================================
        # Phase 2: attention, pair-level software pipeline
        # =====================================================================
        with ExitStack() as ph2:
            x2T = [p_x2t.tile([P, NQ], BF16, tag=f"x2t{kc}", name=f"x2t{kc}")
                   for kc in range(KC)]
            wop = ph2.enter_context(tc.tile_pool(name="wop", bufs=1, side="right"))
            wo = [wstream(wop, Wo_d, mc, f"wom{mc}") for mc in range(KC)]
            pt_pool = ph2.enter_context(tc.tile_pool(name="pt", bufs=3))
            an_pool = ph2.enter_context(tc.tile_pool(name="an", bufs=2))
            sm_pool = ph2.enter_context(tc.tile_pool(name="sm", bufs=2))
            pSC = ph2.enter_context(tc.tile_pool(name="pSC", bufs=2, space="PSUM"))
            pAV2 = ph2.enter_context(tc.tile_pool(name="pAV2", bufs=3, space="PSUM"))
            pWO = ph2.enter_context(tc.tile_pool(name="pWO", bufs=1, space="PSUM"))

            # interleave a mask-heavy slot (0 or 2: half/all of their k-tile
            # groups carry masks -> DVE-heavy rounds) with a mask-light one
            # (1 or 3) so the per-round DVE load stays under the exp pace.
            tasks = []
            for hi, lo in ((1, 0), (3, 2)):
                for m in range(NPAIR):
                    tasks += [(hi, m), (lo, m)]
            task_of = {sm: t for t, sm in enumerate(tasks)}
            state = {}   # task idx -> dict(pt=[...], av=tile, s=, m=)

            def issue_scores(t, kt2):
                s, m = tasks[t]
                psc = pSC.tile([P, 1024], F32, tag="sc", name="psc")
                qe = qT[(m, s)]
                for j in range(2):
                    kws = slice((2 * kt2 + j) * P, (2 * kt2 + j + 1) * P)
                    nc.tensor.matmul(psc[:, j * CW:(j + 1) * CW],
                                     kT[m][0:64, kws], qe[0:64, :],
                                     start=True, stop=True,
                                     skip_group_check=True)
                    nc.tensor.matmul(psc[:, 512 + j * CW:512 + (j + 1) * CW],
                                     kT[m][64:128, kws], qe[64:128, :],
                                     start=True, stop=True,
                                     skip_group_check=True)
                pt = pt_pool.tile([P, 1024], BF16, tag=f"pt{kt2}",
                                  name=f"pt{kt2}")
                nc.scalar.activation(pt[:], psc[:], AF.Exp)
                nkt2 = NKT_PROG[s] // 2
                if kt2 >= nkt2 - 2:
                    g = kt2 - (nkt2 - 2)
                    mt = masks[0][:, s, g, :]
                    nc.vector.tensor_tensor(pt[:, 0:512], pt[:, 0:512], mt,
                                            ALU.mult)
                    nc.vector.tensor_tensor(pt[:, 512:1024], pt[:, 512:1024],
                                            mt, ALU.mult)
                state[t]["pt"].append(pt)

            def issue_av(t, par, kt2):
                # av_e (par=0) and av_o (par=1) share one psum bank, split by
                # column.  A start=True matmul marks its whole 2KB zero-region
                # pending, so the two accumulation groups must NOT interleave:
                # par=0 runs to completion before par=1's start (verified on
                # hw: reads of pending-but-unwritten bytes see old data, but
                # accumulates onto them lose the prior value).
                s, m = tasks[t]
                st = state[t]
                nkt2 = NKT_PROG[s] // 2
                av = st["av"]
                pt = st["pt"][kt2]
                for j in range(2):
                    kt = 2 * kt2 + j
                    b = (kt2 == 0 and j == 0)
                    e = (kt2 == nkt2 - 1 and j == 1)
                    nc.tensor.matmul(av[:, par * CW:(par + 1) * CW],
                                     v65[kt][:, m, par, :],
                                     pt[:, par * 512 + j * CW:
                                        par * 512 + (j + 1) * CW],
                                     start=b, stop=e, skip_group_check=True)

            def issue_normalize(t):
                s, m = tasks[t]
                av = state[t]["av"]
                an = an_pool.tile([P, CW], BF16, tag=f"an{m}", name=f"an{m}")
                state[t]["an"] = an
                den = sm_pool.tile([1, 512], F32, tag="den", name="den")
                nc.vector.tensor_copy(den[:], av[64:65, :])
                rec = sm_pool.tile([1, 512], F32, tag="rec", name="rec")
                nc.vector.reciprocal_approx_fast(rec[:], den[:])
                recb = sm_pool.tile([1, 512], BF16, tag="recb", name="recb")
                nc.vector.tensor_copy(recb[:], rec[:])
                bcb = sm_pool.tile([64, 512], BF16, tag="bcb", name="bcb")
                for par in range(2):
                    cs = slice(par * CW, (par + 1) * CW)
                    nc.gpsimd.partition_broadcast(bcb[:, cs], recb[:, cs])
                nc.vector.tensor_tensor(an[0:64, :], av[0:64, 0:CW],
                                        bcb[:, 0:CW], ALU.mult)
                tmo = sm_pool.tile([64, CW], BF16, tag="tmo", name="tmo")
                nc.vector.tensor_tensor(tmo[:], av[0:64, CW:2 * CW],
                                        bcb[:, CW:2 * CW], ALU.mult)
                nc.sync.dma_start(out=an[64:128, :], in_=tmo[:])

            def issue_wo_chain(mc, s):
                ps = pWO.tile([P, CW], F32, tag="wo", name="wops")
                for k in range(NPAIR):
                    nc.tensor.matmul(ps[:], wo[mc][:, k, :],
                                     state[task_of[(s, k)]]["an"][:],
                                     start=(k == 0), stop=(k == NPAIR - 1))
                nc.vector.tensor_tensor(x2T[mc][:, s * CW:(s + 1) * CW],
                                        ps[:], xq[mc][:, s * CW:(s + 1) * CW],
                                        ALU.add)

            def finish_task(t):
                """AV (odd parity) + normalize for task t; returns Wo work
                if t closed a slot."""
                pn2 = NKT_PROG[tasks[t][0]] // 2
                for kt2 in range(pn2):
                    issue_av(t, 1, kt2)
                issue_normalize(t)
                if tasks[t][1] == NPAIR - 1:
                    return [(mc, tasks[t][0]) for mc in range(KC)]
                return []

            # AV lags scores by TWO tasks so the exp pipeline stays full even
            # on short (nkt=4) slots; Wo chains are spread one-per-round so
            # slot boundaries don't drain the exp stream.
            NT = len(tasks)
            pending_wo = []
            for t in range(NT):
                s, m = tasks[t]
                nkt2 = NKT_PROG[s] // 2
                state[t] = {"pt": [], "av": pAV2.tile([65, 512], F32, tag="av",
                                                      name="av")}
                pn = NKT_PROG[tasks[t - 2][0]] // 2 if t >= 2 else 0
                for kt2 in range(max(nkt2, pn)):
                    if kt2 < nkt2:
                        issue_scores(t, kt2)
                    if t >= 2 and kt2 < pn:
                        issue_av(t - 2, 0, kt2)
                    if pending_wo and kt2 % 2 == 1:
                        issue_wo_chain(*pending_wo.pop(0))
                if t >= 2:
                    pending_wo += finish_task(t - 2)
            for t in (NT - 2, NT - 1):
                for kt2 in range(NKT_PROG[tasks[t][0]] // 2):
                    issue_av(t, 0, kt2)
                pending_wo += finish_task(t)
                while pending_wo:
                    issue_wo_chain(*pending_wo.pop(0))

        # =====================================================================
        # Phase 3: LN2 + FFN (gamma2 folded into W1, ln2_b into b1)
        # =====================================================================
        with ExitStack() as ph5:
            p_ln2T = ph5.enter_context(tc.tile_pool(name="p_ln2T", bufs=1))
            ln2stack = ExitStack()
            lnw2 = ln2stack.enter_context(tc.tile_pool(name="lnw2", bufs=2))
            pstat2 = ln2stack.enter_context(tc.tile_pool(name="pstat2", bufs=2,
                                                         space="PSUM"))
            ln2T = [p_ln2T.tile([P, NQ], BF16, tag=f"ln2T{kc}", name=f"ln2T{kc}")
                    for kc in range(KC)]
            for i in range(NQ // 512):
                sl = slice(i * 512, (i + 1) * 512)
                ps_sum = pstat2.tile([1, 512], F32, tag="lns", name="ps_sum")
                ps_sq = pstat2.tile([1, 512], F32, tag="lnq", name="ps_sq")
                for kc in range(KC):
                    sq = lnw2.tile([P, 512], BF16, tag="sq", name="sq")
                    nc.vector.tensor_tensor(sq[:], x2T[kc][:, sl], x2T[kc][:, sl],
                                            ALU.mult)
                    nc.tensor.matmul(ps_sum[:], onesC[:], x2T[kc][:, sl],
                                     start=(kc == 0), stop=(kc == KC - 1),
                                     skip_group_check=True)
                    nc.tensor.matmul(ps_sq[:], onesC[:], sq[:],
                                     start=(kc == 0), stop=(kc == KC - 1),
                                     skip_group_check=True)
                mu_row = lnw2.tile([1, 512], BF16, tag="mu_row", name="mu_row")
                msq_row = lnw2.tile([1, 512], BF16, tag="msq_row", name="msq_row")
                nc.scalar.activation(mu_row[:], ps_sum[:], AF.Copy, scale=1.0 / C)
                nc.scalar.activation(msq_row[:], ps_sq[:], AF.Copy, scale=1.0 / C)
                mu_b = lnw2.tile([P, 512], BF16, tag="mu_b", name="mu_b")
                nc.gpsimd.partition_broadcast(mu_b[:], mu_row[:])
                msq_b = lnw2.tile([P, 512], BF16, tag="msq_b", name="msq_b")
                nc.gpsimd.partition_broadcast(msq_b[:], msq_row[:])
                var = lnw2.tile([P, 512], F32, tag="var", name="var")
                nc.vector.tensor_tensor(var[:], mu_b[:], mu_b[:], ALU.mult)
                nc.vector.tensor_tensor(var[:], msq_b[:], var[:], ALU.subtract)
                nc.scalar.activation(var[:], var[:], AF.Sqrt, bias=eps_col[:])
                rstd_f = lnw2.tile([P, 512], F32, tag="rstd_f", name="rstd_f")
                nc.vector.reciprocal_approx_fast(rstd_f[:], var[:])
                rstd_b = lnw2.tile([P, 512], BF16, tag="rstd_b", name="rstd_b")
                nc.vector.tensor_copy(rstd_b[:], rstd_f[:])
                for kc in range(KC):
                    nc.vector.tensor_tensor(ln2T[kc][:, sl], x2T[kc][:, sl],
                                            mu_b[:], ALU.subtract)
                    nc.vector.tensor_tensor(ln2T[kc][:, sl], ln2T[kc][:, sl],
                                            rstd_b[:], ALU.mult)
            ln2stack.close()

            ff1_pool = ph5.enter_context(tc.tile_pool(name="ff1", bufs=1))
            facc_pool = ph5.enter_context(tc.tile_pool(name="facc", bufs=1))
            w1_pool = ph5.enter_context(tc.tile_pool(name="w1s", bufs=3))
            w2_pool = ph5.enter_context(tc.tile_pool(name="w2s", bufs=2))
            fst_pool = ph5.enter_context(tc.tile_pool(name="fst", bufs=3))
            pF = ph5.enter_context(tc.tile_pool(name="pF", bufs=4, space="PSUM"))
            ffacc = [facc_pool.tile([P, NQ], BF16, tag=f"facc{m}", name=f"ffacc{m}")
                     for m in range(KC)]
            for half in range(2):
                hoff = half * 2048
                ff1 = []
                for m in range(16):
                    mm = half * 16 + m
                    w1m = w1_pool.tile([P, KC, P], BF16, tag="w1m", name="w1m")
                    nc.sync.dma_start(
                        out=w1m[:],
                        in_=W1_d[:, hoff + m * P: hoff + (m + 1) * P]
                        .rearrange("(kc p) m -> p kc m", p=P))
                    f = ff1_pool.tile([P, NQ], BF16, tag=f"f{m}", name=f"f{m}")
                    for tch in range(2):
                        sl = slice(tch * 512, (tch + 1) * 512)
                        psf = pF.tile([P, 512], F32, tag="proj", name="f1ps")
                        for kc in range(KC):
                            nc.tensor.matmul(psf[:], w1m[:, kc, :], ln2T[kc][:, sl],
                                             start=(kc == 0), stop=(kc == KC - 1))
                        nc.scalar.activation(f[:, sl], psf[:], AF.Relu,
                                             bias=b1t[:, mm:mm + 1])
                    ff1.append(f)
                for mc in range(KC):
                    w2m = w2_pool.tile([P, 16, P], BF16, tag="w2m", name="w2m")
                    nc.sync.dma_start(
                        out=w2m[:],
                        in_=W2_d[hoff:hoff + 2048, mc * P:(mc + 1) * P]
                        .rearrange("(kt p) m -> p kt m", p=P))
                    for tch in range(2):
                        sl = slice(tch * 512, (tch + 1) * 512)
                        psf = pF.tile([P, 512], F32, tag="proj", name="f2ps")
                        for kt in range(16):
                            nc.tensor.matmul(psf[:], w2m[:, kt, :], ff1[kt][:, sl],
                                             start=(kt == 0), stop=(kt == 15))
                        if half == 0:
                            nc.vector.tensor_scalar(ffacc[mc][:, sl], psf[:],
                                                    b2t[:, mc:mc + 1], None,
                                                    ALU.add)
                        else:
                            o = fst_pool.tile([P, 512], F32, tag="fo", name="fo")
                            nc.vector.tensor_tensor(o[:], psf[:], ffacc[mc][:, sl],
                                                    ALU.add)
                            nc.vector.tensor_tensor(o[:], o[:], x2T[mc][:, sl],
                                                    ALU.add)
                            nc.sync.dma_start(out=out_d[mc * P:(mc + 1) * P, sl],
                                              in_=o[:])

    nc.compile()
    return nc, names


_CACHE = {}


def _get_built():
    if "nc" not in _CACHE:
        _CACHE["nc"], _CACHE["names"] = _build()
    return _CACHE["nc"], _CACHE["names"]


def _bf16(a):
    import ml_dtypes
    return np.ascontiguousarray(np.asarray(a).astype(ml_dtypes.bfloat16))


# role-1 cores get xT with adjacent 256-token chunks swapped, so that the
# program's role-0 query columns hold the role-1 chunks.  chunk_at[p] = global
# chunk stored at program chunk position p.
_CHUNK_AT = {0: list(range(8)), 1: [1, 0, 3, 2, 5, 4, 7, 6]}


def _host_inputs(x, Wq, Wk, Wv, Wo, bo, ln1_g, ln1_b, ln2_g, ln2_b, W1, b1, W2, b2):
    """Build the 8 per-core input maps (host work = sharding/layout + affine
    weight folding)."""
    f = np.float32
    g1 = ln1_g.astype(f)[:, None]
    Wq_f = SCALE * g1 * Wq.astype(f)
    Wk_f = g1 * Wk.astype(f)
    Wv_f = g1 * Wv.astype(f)
    bq = SCALE * (ln1_b.astype(f) @ Wq.astype(f))          # q bias (applied)
    bv = ln1_b.astype(f) @ Wv.astype(f)                    # v bias -> folds into bo
    bo_f = bo.astype(f) + bv @ Wo.astype(f)
    g2 = ln2_g.astype(f)[:, None]
    W1_f = g2 * W1.astype(f)
    b1_f = b1.astype(f) + ln2_b.astype(f) @ W1.astype(f)

    shared = {
        "Wq": _bf16(Wq_f), "Wk": _bf16(Wk_f), "Wv": _bf16(Wv_f),
        "Wo": _bf16(Wo.astype(f)), "W1": _bf16(W1_f), "W2": _bf16(W2.astype(f)),
        "bqt": np.ascontiguousarray(bq.reshape(NPAIR, P).T, f),
        "bot": np.ascontiguousarray(bo_f.reshape(KC, P).T, f),
        "b1t": np.ascontiguousarray(b1_f.reshape(32, P).T, f),
        "b2t": np.ascontiguousarray(b2.astype(f).reshape(KC, P).T, f),
        "onesC": _bf16(np.ones((P, 1), f)),
        "onesB": _bf16(np.ones((1, P), f)),
    }
    kl = np.arange(P)[:, None]
    ql = np.arange(CW)[None, :]
    in_maps = []
    for c in range(8):
        b, r = c // 2, c % 2
        chunk_at = _CHUNK_AT[r]
        xTb = np.ascontiguousarray(x[b].T.astype(f))  # (C, T)
        if r == 1:
            cols = np.concatenate([np.arange(CW * pc, CW * (pc + 1))
                                   for pc in chunk_at])
            xTb = np.ascontiguousarray(xTb[:, cols])
        # causal masks in program k-coordinates: program k position j lives in
        # program chunk j//CW, which holds global chunk chunk_at[j//CW]; its
        # global index is chunk_at[j//CW]*CW + j%CW.  Query slot s holds global
        # chunk GSETS[r][s].  Layout: [s, g, 0:256]=tile nkt-4+2g,
        # [s, g, 256:512]=tile nkt-4+2g+1.
        m = np.empty((NSLOT, 2, P, 512), f)
        for s in range(NSLOT):
            q_glob0 = CW * GSETS[r][s]
            for g in range(2):
                for j in range(2):
                    kt = NKT_PROG[s] - 4 + 2 * g + j
                    kpos = P * kt + kl                      # [P,1] program index
                    kglob = (np.asarray(chunk_at)[kpos // CW] * CW) + (kpos % CW)
                    m[s, g, :, j * CW:(j + 1) * CW] = \
                        (kglob <= (q_glob0 + ql)).astype(f)
        im = dict(shared)
        im["xT"] = _bf16(xTb)
        im["masks"] = _bf16(np.ascontiguousarray(m.transpose(2, 0, 1, 3)))
        in_maps.append(im)
    return in_maps


def _unshard(outs):
    out = np.empty((4, T, C), np.float32)
    for c in range(8):
        b, r = c // 2, c % 2
        oT = outs[c]  # (C, NQ), program slot order
        for s in range(NSLOT):
            g = GSETS[r][s]
            out[b, CW * g:CW * (g + 1), :] = oT[:, CW * s:CW * (s + 1)].T
    return out


def kernel(**inputs):
    from concourse.bass_utils import run_bass_kernel_spmd
    from concourse.bass_interp import get_hw_module

    args = {k: np.asarray(v, np.float32) for k, v in inputs.items()}
    in_maps_named = _host_inputs(**args)

    nc, names = _get_built()
    in_maps = [{names[k]: v for k, v in im.items()} for im in in_maps_named]

    hw = get_hw_module(nc.m)
    old = nc.m
    nc.m = hw
    try:
        res = run_bass_kernel_spmd(nc, in_maps, core_ids=list(range(8)))
    finally:
        nc.m = old
    outs = [r[names["out"]] for r in res.results]
    return _unshard(outs)


if __name__ == "__main__":
    import reference
    inp = {k: np.asarray(v) for k, v in reference.setup_inputs().items()}
    got = kernel(**inp)
    exp = np.asarray(reference.reference(**inp))
    err = np.abs(got - exp).max() / np.abs(exp).max()
    print("Relative error:", err)
